# revision 24
# baseline (speedup 1.0000x reference)
"""Trainium2 Bass kernel for nn_AttentionBlock (GroupNorm + single-head
self-attention + proj + residual), data-parallel over batch on 8 cores.

Contract: kernel(**inputs) takes the FULL unsharded inputs
  x (8, 256, 64, 64) f32, gn_scale (256,), gn_bias (256,),
  qkv_w (768, 256), qkv_b (768,), proj_w (256, 256), proj_b (256,)
and returns the FULL output (8, 256, 64, 64) f32.

v2 design (from the v1 NTFF trace: PE 90% busy on matmuls, ACT co-bound
on exp, DVE saturated by denominator accumulation):
  - GroupNorm folded into the QKV weights: w_eff[c,o] = 32*W[o,c]*m_c on
    device (m_c = rstd*gamma per channel), so no xn tensor is ever
    materialized. The additive GN term (a_c) becomes per-output biases
    via tiny matmuls (W@a). x is shipped from host in BOTH f32 (GN stats
    + residual) and fp8 (QKV matmul operand).
  - QKV/scores/PV all fp8 DoubleRow (K=256 in one pass).
  - Softmax denominator on the PE: a ones-lhsT DR matmul per key-block
    pair accumulates den[q] into the same PSUM tile group as the PV
    output (tile [P, 3, 512]: ch0, ch1, den) -> zero DVE work in the
    steady loop.
  - Steady state per 512-q-tile step: PE 5 matmuls (2 scores, 2 PV,
    1 den) ~1.1us; ACT one 1024-wide exp ~1.1us. PV/den run one step
    behind scores so ACT never waits on PE.
  - PSUM banks: scores 2x[P,2,512]=4, out+den [P,3,512]=3, proj 1 = 8.
"""

import os
import sys

import numpy as np

for _p in (
    "/opt/trn_rl_repo",
    "/root/.axon_site",
    "/root/.axon_site/_ro/trn_rl_repo",
    "/root/.axon_site/_ro/pypackages",
):
    if os.path.isdir(_p) and _p not in sys.path:
        sys.path.append(_p)

import ml_dtypes  # noqa: E402

import concourse.bass as bass  # noqa: E402
import concourse.mybir as mybir  # noqa: E402
import concourse.tile as tile  # noqa: E402
from concourse import bacc  # noqa: E402

F32 = mybir.dt.float32
BF16 = mybir.dt.bfloat16
FP8 = mybir.dt.float8e4
AF = mybir.ActivationFunctionType
ALU = mybir.AluOpType
DR = mybir.MatmulPerfMode.DoubleRow

B, C, H, W = 8, 256, 64, 64
GROUPS = 8
EPS = 1e-5
P = 128
N_CORES = 8
ATT_SCALE = float(C) ** -0.5  # 1/16
WS = 32.0                     # host pre-scale on fp8 qkv weights
INV_WS = 1.0 / WS


def build_nc(n_tok=H * W):
    """Build the single-core Bass program (SPMD across 8 cores)."""
    CCH = C // P            # channel chunks (2)
    QT = 512                # q-tile width (one PSUM bank of f32)
    NQ = n_tok // QT        # number of q tiles (8)
    NKB = n_tok // P        # number of 128-token key blocks (32)
    NKP = NKB // 2          # key-block pairs per q tile (16)
    GSZ = C // GROUPS       # channels per group (32)

    nc = bacc.Bacc()

    # ---- DRAM I/O (per-core tensors; host shards batch over cores) ----
    x_d = nc.dram_tensor("x", [C, n_tok], F32, kind="ExternalInput")
    xf8_d = nc.dram_tensor("x_f8", [CCH, P, n_tok], FP8, kind="ExternalInput")
    # token-major x and x^2 in fp8 for PE-computed GN statistics
    NKB = n_tok // P
    xst_d = nc.dram_tensor("x_st", [2, NKB, P, C], FP8, kind="ExternalInput")
    qkvw_d = nc.dram_tensor("qkv_wt", [CCH, P, 3 * C], FP8, kind="ExternalInput")
    vbias_d = nc.dram_tensor("v_bias", [C], F32, kind="ExternalInput")
    projw_d = nc.dram_tensor("proj_wt", [CCH, P, C], BF16, kind="ExternalInput")
    # packed per-partition scalars: qkb(4), projb(2), gnsc(2), gnbi(2)
    smalls_d = nc.dram_tensor("smalls", [10, P], F32, kind="ExternalInput")
    # packed indicators: [0] = gn_ind (c -> group), [1] = gn_ind2 (group -> c)
    inds_d = nc.dram_tensor("gn_inds", [2, CCH, P, P], F32, kind="ExternalInput")
    out_d = nc.dram_tensor("out", [C, n_tok], F32, kind="ExternalOutput")

    with tile.TileContext(nc) as tc:
        with (
            tc.tile_pool(name="persist", bufs=1) as pp,
            tc.tile_pool(name="work", bufs=3) as wp,
            tc.tile_pool(name="ps", bufs=1, space="PSUM") as psp,
        ):
            # ---------------- DMAs: stats inputs first ----------------
            # token-major x and x^2 (fp8): feed the PE-computed GN sums
            xt_sb = pp.tile([P, 2, NKB, C], FP8, tag="xt_sb")
            for w in range(2):
                nc.sync.dma_start(
                    xt_sb[:, w],
                    xst_d.rearrange("w b p c -> p w b c")[:, w],
                )
            x_f8 = pp.tile([P, CCH, n_tok], FP8, tag="x_f8")
            XPC = 4
            for pc in range(XPC):
                xs = slice(pc * (n_tok // XPC), (pc + 1) * (n_tok // XPC))
                nc.sync.dma_start(
                    x_f8[:, :, xs],
                    xf8_d.rearrange("t p n -> p t n")[:, :, xs],
                )
            qkvw = pp.tile([P, CCH, 3 * C], FP8, tag="qkvw")
            nc.sync.dma_start(qkvw[:], qkvw_d.rearrange("t p o -> p t o"))
            smalls = pp.tile([P, 10], F32, tag="smalls")
            nc.sync.dma_start(smalls[:], smalls_d.rearrange("j p -> p j"))
            qkb = smalls[:, 0:4]
            projb = smalls[:, 4:6]
            gnsc = smalls[:, 6:8]
            gnbi = smalls[:, 8:10]
            inds = pp.tile([P, 2, CCH, P], F32, tag="inds")
            nc.sync.dma_start(inds[:], inds_d.rearrange("w t p g -> p w t g"))
            gnind2 = inds[:, 1]
            projw = pp.tile([P, CCH, C], BF16, tag="projw")
            nc.sync.dma_start(projw[:], projw_d.rearrange("t p o -> p t o"))
            # V bias broadcast across partitions (DMA with partition-stride 0)
            vbias = pp.tile([P, C], F32, tag="vbias")
            nc.sync.dma_start(vbias[:], vbias_d[None, :].to_broadcast([P, C]))
            # ones for the denominator matmul (fp8, DR: [K=128, 2, M=128])
            ones_f8 = pp.tile([P, 2, P], FP8, tag="ones_f8")
            nc.vector.memset(ones_f8[:], 1.0)
            # single-partition ones for broadcast / transpose matmuls
            onescol = pp.tile([1, P], F32, tag="onescol")
            nc.vector.memset(onescol[:], 1.0)

            # ---------------- GN stats on the PE ----------------
            # sum_t x[c, t] and sum_t x^2[c, t] via ones-lhsT DR matmuls on
            # the token-major fp8 tensors -> two [1, 256] psum rows; tiny
            # K=1 matmuls transpose them back onto channel partitions.
            stats = pp.tile([P, CCH, 2], F32, tag="stats")
            strow = pp.tile([1, 2, C], F32, tag="strow")
            for w in range(2):
                srow_ps = psp.tile([P, QT], F32, tag=("p" if w == 0 else "o"),
                                   name=f"srow{w}")
                for b in range(NKB // 2):
                    nc.tensor.matmul(
                        srow_ps[:1, :C],
                        ones_f8[:, :, 0:1],
                        xt_sb[:, w, 2 * b:2 * b + 2],
                        start=(b == 0),
                        stop=(b == NKB // 2 - 1),
                        perf_mode=DR,
                    )
                nc.vector.tensor_copy(strow[:, w], srow_ps[:1, :C])
            for w in range(2):
                for t in range(CCH):
                    stT_ps = psp.tile([P, QT], F32,
                                      tag=("p" if (2 * w + t) % 2 == 0 else "o"),
                                      name=f"stT{w}{t}")
                    nc.tensor.matmul(
                        stT_ps[:, 0:1],
                        strow[:1, w, t * P:(t + 1) * P],
                        onescol[:, 0:1],
                        start=True,
                        stop=True,
                    )
                    nc.vector.tensor_copy(stats[:, t, w, None], stT_ps[:, 0:1])
            # f32 x for the residual: queued last, consumed from finalize()
            # well into the attention phase.
            x_sb = pp.tile([P, CCH, n_tok], F32, tag="x_sb")
            for t in range(CCH):
                for pc in range(XPC):
                    xs = slice(pc * (n_tok // XPC), (pc + 1) * (n_tok // XPC))
                    nc.sync.dma_start(x_sb[:, t, xs], x_d[t * P:(t + 1) * P, xs])

            # group aggregation: gagg[g, j] = sum_{c in group g} stats[c, j]
            gagg_ps = psp.tile([P, QT], F32, tag="p", name="gagg_ps")
            for t in range(CCH):
                nc.tensor.matmul(
                    gagg_ps[:, :2],
                    inds[:, 0, t],
                    stats[:, t],
                    start=(t == 0),
                    stop=(t == CCH - 1),
                )
            # per-group a = rstd, b = -mean * rstd  (sums / (GSZ * n_tok))
            gab = pp.tile([P, 2], F32, tag="gab")
            nc.vector.memset(gab[:], 0.0)
            gmean = wp.tile([P, 1], F32, tag="gmean")
            gtmp = wp.tile([P, 1], F32, tag="gtmp")
            nc.vector.tensor_scalar_mul(gmean[:GROUPS], gagg_ps[:GROUPS, 0:1],
                                        1.0 / (GSZ * n_tok))
            nc.vector.tensor_scalar_mul(gtmp[:GROUPS], gagg_ps[:GROUPS, 1:2],
                                        1.0 / (GSZ * n_tok))
            # gtmp := mean^2 - E[x^2] = -var
            nc.vector.scalar_tensor_tensor(
                out=gtmp[:GROUPS],
                in0=gmean[:GROUPS],
                scalar=gmean[:GROUPS],
                in1=gtmp[:GROUPS],
                op0=ALU.mult,
                op1=ALU.subtract,
            )
            # std = sqrt(-1 * gtmp + eps)
            epsb = wp.tile([P, 1], F32, tag="epsb")
            nc.vector.memset(epsb[:], EPS)
            nc.scalar.activation(gtmp[:GROUPS], gtmp[:GROUPS], AF.Sqrt,
                                 bias=epsb[:GROUPS], scale=-1.0)
            nc.vector.reciprocal(gab[:GROUPS, 0:1], gtmp[:GROUPS])  # a = rstd
            nc.vector.tensor_mul(gtmp[:GROUPS], gmean[:GROUPS], gab[:GROUPS, 0:1])
            nc.vector.tensor_scalar_mul(gab[:GROUPS, 1:2], gtmp[:GROUPS], -1.0)

            # broadcast (a, b) to channels; fold GN into the fp8 weights:
            #   m_c = rstd_g * gamma_c ; a_c = (-mean*rstd)*gamma_c + beta_c
            #   w_eff[c, o] = qkvw[c, o] * m_c        (qkvw = 32*W^T)
            #   a2_c = 32 * a_c / m_c   (fp8; a-term via w_eff @ a2 / 1024)
            w_eff = pp.tile([P, CCH, 3 * C], FP8, tag="w_eff")
            a_col = pp.tile([P, CCH, 1], FP8, tag="a_col")
            chms = []
            for t in range(CCH):
                chab_ps = psp.tile([P, QT], F32, tag="p", name=f"chab_ps{t}")
                nc.tensor.matmul(chab_ps[:, :2], gnind2[:, t], gab[:],
                                 start=True, stop=True)
                chm = pp.tile([P, 1], F32, tag=f"chm{t}", name=f"chm{t}")
                cha = pp.tile([P, 1], F32, tag=f"cha{t}", name=f"cha{t}")
                nc.vector.tensor_mul(chm[:], chab_ps[:, 0:1], gnsc[:, t, None])
                nc.vector.scalar_tensor_tensor(
                    out=cha[:],
                    in0=chab_ps[:, 1:2],
                    scalar=gnsc[:, t, None],
                    in1=gnbi[:, t, None],
                    op0=ALU.mult,
                    op1=ALU.add,
                )
                chms.append(chm)
                nc.vector.tensor_scalar_mul(w_eff[:, t], qkvw[:, t], chm[:])
                # a2 = 32 * cha / chm  (fp8)
                rchm = wp.tile([P, 1], F32, tag=f"rchm{t}", name=f"rchm{t}")
                nc.vector.reciprocal(rchm[:], chm[:])
                nc.vector.scalar_tensor_tensor(
                    out=a_col[:, t],
                    in0=cha[:],
                    scalar=WS,
                    in1=rchm[:],
                    op0=ALU.mult,
                    op1=ALU.mult,
                )

            # ---- bias vectors: Wa terms via tiny matmuls ----
            # Q/K: qkb_eff[d, j] = qkb[d, j] + (W@a)[j*128+d] / 1024
            qkb_eff = pp.tile([P, 4], F32, tag="qkb_eff")
            for j in range(4):
                wa_ps = psp.tile([P, QT], F32, tag="p", name=f"wa_ps{j}")
                for t in range(CCH):
                    nc.tensor.matmul(
                        wa_ps[:, 0:1],
                        w_eff[:, t, j * P:(j + 1) * P],
                        a_col[:, t],
                        start=(t == 0),
                        stop=(t == CCH - 1),
                    )
                nc.vector.scalar_tensor_tensor(
                    out=qkb_eff[:, j, None],
                    in0=wa_ps[:, 0:1],
                    scalar=1.0 / (WS * WS),
                    in1=qkb[:, j, None],
                    op0=ALU.mult,
                    op1=ALU.add,
                )
            # V: vb_eff[*, d] = vbias[d] + (Wv@a)[d] / 1024, broadcast to all
            # partitions via a ones-column fp32 matmul.
            vr_ps = psp.tile([P, QT], F32, tag="p", name="vr_ps")
            for t in range(CCH):
                nc.tensor.matmul(
                    vr_ps[:1, :C],
                    a_col[:, t],
                    w_eff[:, t, 2 * C:3 * C],
                    start=(t == 0),
                    stop=(t == CCH - 1),
                )
            vrow = pp.tile([1, C], F32, tag="vrow")
            nc.vector.tensor_copy(vrow[:], vr_ps[:1, :C])
            vb_ps = psp.tile([P, QT], F32, tag="p", name="vb_ps")
            nc.tensor.matmul(vb_ps[:, :C], onescol[:], vrow[:],
                             start=True, stop=True)
            vb_eff = pp.tile([P, C], F32, tag="vb_eff")
            nc.vector.scalar_tensor_tensor(
                out=vb_eff[:],
                in0=vb_ps[:, :C],
                scalar=1.0 / (WS * WS),
                in1=vbias[:],
                op0=ALU.mult,
                op1=ALU.add,
            )

            # ---------------- QKV ----------------
            # Q, K in (d, n) fp8; V token-major fp8. All matmuls fp8 DR
            # (K=256 contraction in one pass). Only the blocks the first
            # attention steps need are emitted up front (copies on ACT,
            # which is otherwise idle before the first exp); the rest are
            # deadline-scheduled INTO the attention loop with copies on the
            # DVE, which is idle during attention.
            qk = pp.tile([P, 4, n_tok], FP8, tag="qk")
            v_sb = pp.tile([P, NKB, C], FP8, tag="v_sb")

            import itertools
            _tag_cycle = itertools.cycle([("s", 2), ("o", 1), ("s", 2), ("p", 1)])

            def emit_qk_half(j, h, engine, tag=None):
                # one [P, 512] half-block of Q (j<2) or K (j>=2). Upfront
                # (pre-attention) tiles cycle across all psum tags; in-loop
                # Q halves ride the "s" rotation (transient, DVE-copied).
                ns = slice(h * QT, (h + 1) * QT)
                tg, bf = tag if tag else ("s", 2)
                qp = psp.tile([P, QT], F32, tag=tg, bufs=bf,
                              name=f"qp{j}_{h}")
                nc.tensor.matmul(
                    qp[:],
                    w_eff[:, :, j * P:(j + 1) * P],
                    x_f8[:, :, ns],
                    start=True,
                    stop=True,
                    perf_mode=DR,
                )
                if engine == "act":
                    nc.scalar.activation(
                        qk[:, j, ns], qp[:],
                        AF.Identity,
                        bias=qkb_eff[:, j, None],
                        scale=INV_WS,
                    )
                else:
                    nc.vector.tensor_scalar(
                        out=qk[:, j, ns],
                        in0=qp[:],
                        scalar1=INV_WS,
                        scalar2=qkb_eff[:, j, None],
                        op0=ALU.mult,
                        op1=ALU.add,
                    )

            def emit_v_block(tp, engine="dve", tag=None):
                # two 128-token blocks of V: out [tok, 2, C]
                tg, bf = tag if tag else ("s", 2)
                vp = psp.tile([P, 2, C], F32, tag=tg, bufs=bf, name=f"vp{tp}")
                for k2 in range(2):
                    tb = 2 * tp + k2
                    nc.tensor.matmul(
                        vp[:, k2],
                        x_f8[:, :, tb * P:(tb + 1) * P],
                        w_eff[:, :, 2 * C:3 * C],
                        start=True,
                        stop=True,
                        perf_mode=DR,
                    )
                nc.vector.scalar_tensor_tensor(
                    out=v_sb[:, 2 * tp:2 * tp + 2],
                    in0=vp[:],
                    scalar=INV_WS,
                    in1=vb_eff[:, None, :].to_broadcast([P, 2, C]),
                    op0=ALU.mult,
                    op1=ALU.add,
                )

            # All of V and K plus Q half 0 run before the attention loop,
            # PSUM tiles cycling over every tag so no single bank chain
            # serializes. ACT gets the early-needed K halves (it is idle
            # until the first exp); DVE takes V and the late K halves,
            # interleaved by deadline.
            for j in (2, 3, 0, 1):
                emit_qk_half(j, 0, "act", next(_tag_cycle))
            for h in (1, 2, 3):
                for j in (2, 3):
                    emit_qk_half(j, h, "act", next(_tag_cycle))
            _kq = [(j, h) for h in (4, 5, 6, 7) for j in (2, 3)]
            for r in range(16):
                emit_v_block(r, "dve", next(_tag_cycle))
                if r % 2 == 1 and r // 2 < len(_kq):
                    j, h = _kq[r // 2]
                    emit_qk_half(j, h, "dve", next(_tag_cycle))

            # Q halves 1-7 are deadline-scheduled into the attention loop
            # (needed at q-tile h = step 16h); copies on the idle DVE,
            # placed mid-qt to stay clear of the boundary work.
            pending = {}

            def sched(step, fn):
                pending.setdefault(step, []).append(fn)

            for h in range(1, 8):
                for j in (0, 1):
                    sched(16 * h - 5 + j,
                          lambda j=j, h=h: emit_qk_half(j, h, "dve"))

            # ---------------- attention ----------------
            # global steps g = qt*NKP + i ; per step:
            #   scores(g):  2 DR matmuls -> s_ps [P, 2(kb), 512]
            #   exp(g):     1 ACT instr [P, 1024] -> pt fp8
            #   pv_den(g-2): 2 PV DR matmuls + 1 ones-DR matmul into
            #                o tile [P, 3, 512] (ch0, ch1, den)
            # PV runs TWO steps behind scores so the PE never waits on the
            # scores->exp->pt chain (exp latency > PE slack per step).
            # finalize is split: rec/obs (DVE) emit at (qt, 2) BEFORE
            # pv_den(qt, 0) so the o-tile reuse is ordered; proj/res emit
            # at (qt, 3).
            o_tiles = {}
            pt_tiles = {}

            def emit_scores_exp(g):
                qt, i = divmod(g, NKP)
                qs = slice(qt * QT, (qt + 1) * QT)
                s_ps = psp.tile([P, 2, QT], F32, tag="s", bufs=2,
                                name=f"s_{g}")
                for k2 in range(2):
                    kb = 2 * i + k2
                    nc.tensor.matmul(
                        s_ps[:, k2],
                        qk[:, 2:4, kb * P:(kb + 1) * P],
                        qk[:, 0:2, qs],
                        start=True,
                        stop=True,
                        perf_mode=DR,
                    )
                pt = wp.tile([P, 2, QT], FP8, tag="pt", bufs=6, name=f"pt_{g}")
                nc.scalar.activation(
                    pt.rearrange("p a b -> p (a b)"),
                    s_ps.rearrange("p a b -> p (a b)"),
                    AF.Exp, scale=ATT_SCALE)
                pt_tiles[g] = pt

            def emit_pv_den(g):
                qt, i = divmod(g, NKP)
                if i == 0:
                    o_tiles[qt] = psp.tile([P, 3, QT], F32, tag="o",
                                           name=f"o_{qt}")
                o = o_tiles[qt]
                pt = pt_tiles.pop(g)
                for ch in range(2):
                    nc.tensor.matmul(
                        o[:, ch],
                        v_sb[:, 2 * i:2 * i + 2, ch * P:(ch + 1) * P],
                        pt[:],
                        start=(i == 0),
                        stop=(i == NKP - 1),
                        perf_mode=DR,
                    )
                nc.tensor.matmul(
                    o[:, 2],
                    ones_f8[:],
                    pt[:],
                    start=(i == 0),
                    stop=(i == NKP - 1),
                    perf_mode=DR,
                )

            def finalize_a(qt):
                # UN-normalized bf16 copies of the PV output (frees the o
                # tile without waiting on the reciprocal chain) + the
                # denominator reciprocal. Normalization happens after proj:
                # proj is linear, so proj(out/den) == proj(out)/den.
                o = o_tiles.pop(qt)
                obs = wp.tile([P, 2, QT], BF16, tag="obs", bufs=2,
                              name=f"obs{qt}")
                for ch in range(2):
                    nc.vector.tensor_copy(obs[:, ch], o[:, ch])
                rec = wp.tile([P, QT], F32, tag="rec", bufs=2, name=f"rec{qt}")
                nc.vector.reciprocal_approx_fast(rec[:], o[:, 2])
                return obs, rec

            def finalize_b(qt, obs, rec, t):
                # one output-channel-chunk of proj + residual + store
                qs = slice(qt * QT, (qt + 1) * QT)
                p_ps = psp.tile([P, QT], F32, tag="p", name=f"pp_{qt}_{t}")
                nc.tensor.matmul(p_ps[:],
                                 projw[:, 0, t * P:(t + 1) * P],
                                 obs[:, 0], start=True, stop=False)
                nc.tensor.matmul(p_ps[:],
                                 projw[:, 1, t * P:(t + 1) * P],
                                 obs[:, 1], start=False, stop=True)
                tmp = wp.tile([P, QT], F32, tag="tmp", bufs=2,
                              name=f"tmp{qt}_{t}")
                nc.vector.tensor_mul(tmp[:], p_ps[:], rec[:])
                res = wp.tile([P, QT], F32, tag="res", bufs=4,
                              name=f"res{qt}_{t}")
                nc.vector.scalar_tensor_tensor(
                    out=res[:],
                    in0=tmp[:],
                    scalar=projb[:, t, None],
                    in1=x_sb[:, t, qs],
                    op0=ALU.add,
                    op1=ALU.add,
                )
                nc.sync.dma_start(out_d[t * P:(t + 1) * P, qs], res[:])

            NST = NQ * NKP
            obs_pending = None
            for g in range(NST):
                qt, i = divmod(g, NKP)
                emit_scores_exp(g)
                for fn in pending.pop(g, ()):
                    fn()
                if qt > 0 and i == 2:
                    obs_pending = (qt - 1,) + finalize_a(qt - 1)
                if g > 1:
                    emit_pv_den(g - 2)
                if qt > 0 and i == 5:
                    finalize_b(*obs_pending, t=0)
                if qt > 0 and i == 9:
                    finalize_b(*obs_pending, t=1)
                    obs_pending = None
            emit_pv_den(NST - 2)
            emit_pv_den(NST - 1)
            oq, obs_l, rec_l = (NQ - 1,) + finalize_a(NQ - 1)
            finalize_b(oq, obs_l, rec_l, t=0)
            finalize_b(oq, obs_l, rec_l, t=1)

    nc.finalize()
    return nc


# ---------------------------------------------------------------------------
# host side
# ---------------------------------------------------------------------------

def _prep_core_inputs(inputs, n_tok=H * W):
    """Build the per-core in_maps (shared weight tensors + per-core x)."""
    CCH = C // P
    f32 = np.float32
    bf16 = ml_dtypes.bfloat16
    fp8 = mybir.dt.np(FP8)

    x = np.asarray(inputs["x"], f32).reshape(B, C, n_tok)
    gn_scale = np.asarray(inputs["gn_scale"], f32)
    gn_bias = np.asarray(inputs["gn_bias"], f32)
    qkv_w = np.asarray(inputs["qkv_w"], f32)
    qkv_b = np.asarray(inputs["qkv_b"], f32)
    proj_w = np.asarray(inputs["proj_w"], f32)
    proj_b = np.asarray(inputs["proj_b"], f32)

    qkv_wt = (np.ascontiguousarray(qkv_w.T) * WS).reshape(CCH, P, 3 * C).astype(fp8)
    v_bias = qkv_b[2 * C:].astype(f32)
    proj_wt = np.ascontiguousarray(proj_w.T).reshape(CCH, P, C).astype(bf16)

    # packed per-partition scalars: qkb(4), projb(2), gnsc(2), gnbi(2)
    smalls = np.concatenate([
        qkv_b[:2 * C].reshape(4, P),
        proj_b.reshape(CCH, P),
        gn_scale.reshape(CCH, P),
        gn_bias.reshape(CCH, P),
    ], axis=0).astype(f32)

    ch = np.arange(C)
    gn_ind = np.zeros((CCH, P, P), f32)
    gn_ind[ch // P, ch % P, ch // (C // GROUPS)] = 1.0
    gn_ind2 = np.zeros((CCH, P, P), f32)
    for t in range(CCH):
        gn_ind2[t, :GROUPS, :] = gn_ind[t, :, :GROUPS].T
    gn_inds = np.stack([gn_ind, gn_ind2])

    shared = {
        "qkv_wt": qkv_wt,
        "v_bias": v_bias,
        "proj_wt": proj_wt,
        "smalls": smalls,
        "gn_inds": gn_inds,
    }
    NKB = n_tok // P
    x_f8 = x.reshape(B, CCH, P, n_tok).astype(fp8)
    # token-major x and x^2 for the PE-computed GN stats
    xt = x.reshape(B, C, NKB, P).transpose(0, 2, 3, 1)          # [B, b, p, c]
    x_st = np.stack([xt, xt * xt], axis=1).astype(fp8)          # [B, 2, b, p, c]
    return [
        dict(shared, x=np.ascontiguousarray(x[i]),
             x_f8=np.ascontiguousarray(x_f8[i]),
             x_st=np.ascontiguousarray(x_st[i]))
        for i in range(B)
    ]


_NC_CACHE = {}
LAST_RESULT = None  # BassKernelResults of the most recent run (for test.py)


def _get_nc():
    if "nc" not in _NC_CACHE:
        _NC_CACHE["nc"] = build_nc()
    return _NC_CACHE["nc"]


def kernel(**inputs) -> np.ndarray:
    global LAST_RESULT
    from concourse.bass_utils import run_bass_kernel_spmd

    nc = _get_nc()
    in_maps = _prep_core_inputs(inputs)
    res = run_bass_kernel_spmd(nc, in_maps, list(range(N_CORES)))
    LAST_RESULT = res
    out = np.stack([np.asarray(res.results[i]["out"]) for i in range(B)])
    return out.reshape(B, C, H, W).astype(np.float32)


# revision 36
# speedup vs baseline: 1.0107x; 1.0107x over previous
"""Trainium2 Bass kernel for nn_AttentionBlock (GroupNorm + single-head
self-attention + proj + residual), data-parallel over batch on 8 cores.

Contract: kernel(**inputs) takes the FULL unsharded inputs
  x (8, 256, 64, 64) f32, gn_scale (256,), gn_bias (256,),
  qkv_w (768, 256), qkv_b (768,), proj_w (256, 256), proj_b (256,)
and returns the FULL output (8, 256, 64, 64) f32.

v2 design (from the v1 NTFF trace: PE 90% busy on matmuls, ACT co-bound
on exp, DVE saturated by denominator accumulation):
  - GroupNorm folded into the QKV weights: w_eff[c,o] = 32*W[o,c]*m_c on
    device (m_c = rstd*gamma per channel), so no xn tensor is ever
    materialized. The additive GN term (a_c) becomes per-output biases
    via tiny matmuls (W@a). x is shipped from host in BOTH f32 (GN stats
    + residual) and fp8 (QKV matmul operand).
  - QKV/scores/PV all fp8 DoubleRow (K=256 in one pass).
  - Softmax denominator on the PE: a ones-lhsT DR matmul per key-block
    pair accumulates den[q] into the same PSUM tile group as the PV
    output (tile [P, 3, 512]: ch0, ch1, den) -> zero DVE work in the
    steady loop.
  - Steady state per 512-q-tile step: PE 5 matmuls (2 scores, 2 PV,
    1 den) ~1.1us; ACT one 1024-wide exp ~1.1us. PV/den run one step
    behind scores so ACT never waits on PE.
  - PSUM banks: scores 2x[P,2,512]=4, out+den [P,3,512]=3, proj 1 = 8.
"""

import os
import sys

import numpy as np

for _p in (
    "/opt/trn_rl_repo",
    "/root/.axon_site",
    "/root/.axon_site/_ro/trn_rl_repo",
    "/root/.axon_site/_ro/pypackages",
):
    if os.path.isdir(_p) and _p not in sys.path:
        sys.path.append(_p)

import ml_dtypes  # noqa: E402

import concourse.bass as bass  # noqa: E402
import concourse.mybir as mybir  # noqa: E402
import concourse.tile as tile  # noqa: E402
from concourse import bacc  # noqa: E402

F32 = mybir.dt.float32
BF16 = mybir.dt.bfloat16
FP8 = mybir.dt.float8e4
AF = mybir.ActivationFunctionType
ALU = mybir.AluOpType
DR = mybir.MatmulPerfMode.DoubleRow

B, C, H, W = 8, 256, 64, 64
GROUPS = 8
EPS = 1e-5
P = 128
N_CORES = 8
ATT_SCALE = float(C) ** -0.5  # 1/16
WS = 32.0                     # host pre-scale on fp8 qkv weights
INV_WS = 1.0 / WS


def build_nc(n_tok=H * W):
    """Build the single-core Bass program (SPMD across 8 cores)."""
    CCH = C // P            # channel chunks (2)
    QT = 512                # q-tile width (one PSUM bank of f32)
    NQ = n_tok // QT        # number of q tiles (8)
    NKB = n_tok // P        # number of 128-token key blocks (32)
    NKP = NKB // 2          # key-block pairs per q tile (16)
    GSZ = C // GROUPS       # channels per group (32)

    nc = bacc.Bacc()

    # ---- DRAM I/O (per-core tensors; host shards batch over cores) ----
    x_d = nc.dram_tensor("x", [C, n_tok], F32, kind="ExternalInput")
    xf8_d = nc.dram_tensor("x_f8", [CCH, P, n_tok], FP8, kind="ExternalInput")
    qkvw_d = nc.dram_tensor("qkv_wt", [CCH, P, 3 * C], FP8, kind="ExternalInput")
    vbias_d = nc.dram_tensor("v_bias", [C], F32, kind="ExternalInput")
    projw_d = nc.dram_tensor("proj_wt", [CCH, P, C], BF16, kind="ExternalInput")
    # packed per-partition scalars: qkb(4), projb(2), gnsc(2), gnbi(2)
    smalls_d = nc.dram_tensor("smalls", [10, P], F32, kind="ExternalInput")
    # packed indicators: [0] = gn_ind (c -> group), [1] = gn_ind2 (group -> c)
    inds_d = nc.dram_tensor("gn_inds", [2, CCH, P, P], F32, kind="ExternalInput")
    out_d = nc.dram_tensor("out", [C, n_tok], F32, kind="ExternalOutput")

    with tile.TileContext(nc) as tc:
        with (
            tc.tile_pool(name="persist", bufs=1) as pp,
            tc.tile_pool(name="work", bufs=3) as wp,
            tc.tile_pool(name="ps", bufs=1, space="PSUM") as psp,
        ):
            # ---------------- DMAs: x_f8 first (stats + QKV input) --------
            x_f8 = pp.tile([P, CCH, n_tok], FP8, tag="x_f8")
            XPC = 4
            for pc in range(XPC):
                xs = slice(pc * (n_tok // XPC), (pc + 1) * (n_tok // XPC))
                nc.sync.dma_start(
                    x_f8[:, :, xs],
                    xf8_d.rearrange("t p n -> p t n")[:, :, xs],
                )
            qkvw = pp.tile([P, CCH, 3 * C], FP8, tag="qkvw")
            nc.sync.dma_start(qkvw[:], qkvw_d.rearrange("t p o -> p t o"))
            smalls = pp.tile([P, 10], F32, tag="smalls")
            nc.sync.dma_start(smalls[:], smalls_d.rearrange("j p -> p j"))
            qkb = smalls[:, 0:4]
            projb = smalls[:, 4:6]
            gnsc = smalls[:, 6:8]
            gnbi = smalls[:, 8:10]
            inds = pp.tile([P, 2, CCH, P], F32, tag="inds")
            nc.sync.dma_start(inds[:], inds_d.rearrange("w t p g -> p w t g"))
            gnind2 = inds[:, 1]
            projw = pp.tile([P, CCH, C], BF16, tag="projw")
            nc.sync.dma_start(projw[:], projw_d.rearrange("t p o -> p t o"))
            # V bias broadcast across partitions (DMA with partition-stride 0)
            vbias = pp.tile([P, C], F32, tag="vbias")
            nc.sync.dma_start(vbias[:], vbias_d[None, :].to_broadcast([P, C]))
            # ones for the denominator matmul (fp8, DR: [K=128, 2, M=128])
            ones_f8 = pp.tile([P, 2, P], FP8, tag="ones_f8")
            nc.vector.memset(ones_f8[:], 1.0)
            # single-partition ones for broadcast / transpose matmuls
            onescol = pp.tile([1, P], F32, tag="onescol")
            nc.vector.memset(onescol[:], 1.0)

            # ---------------- GN stats (bn_stats on the fp8 x) ------------
            # quantization noise on mean/var of 128k samples is ~1e-4
            # relative -- irrelevant against the 2e-2 budget.
            stats = pp.tile([P, CCH, 2], F32, tag="stats")
            for t in range(CCH):
                bn6 = wp.tile([P, n_tok // 512, 6], F32, tag="bn6")
                xv = x_f8[:, t].rearrange("p (a b) -> p a b", b=512)
                for a in range(n_tok // 512):
                    nc.vector.bn_stats(bn6[:, a], xv[:, a])
                nc.vector.bn_aggr(stats[:, t], bn6[:])
                # stats col1 := mean^2 + var = E[x^2] (col0 stays mean)
                nc.vector.scalar_tensor_tensor(
                    out=stats[:, t, 1:2],
                    in0=stats[:, t, 0:1],
                    scalar=stats[:, t, 0:1],
                    in1=stats[:, t, 1:2],
                    op0=ALU.mult,
                    op1=ALU.add,
                )
            # f32 x for the residual: queued last, consumed from finalize()
            # well into the attention phase.
            x_sb = pp.tile([P, CCH, n_tok], F32, tag="x_sb")
            for t in range(CCH):
                for pc in range(XPC):
                    xs = slice(pc * (n_tok // XPC), (pc + 1) * (n_tok // XPC))
                    nc.sync.dma_start(x_sb[:, t, xs], x_d[t * P:(t + 1) * P, xs])

            # group aggregation: gagg[g, j] = sum_{c in group g} stats[c, j]
            gagg_ps = psp.tile([P, QT], F32, tag="p", name="gagg_ps")
            for t in range(CCH):
                nc.tensor.matmul(
                    gagg_ps[:, :2],
                    inds[:, 0, t],
                    stats[:, t],
                    start=(t == 0),
                    stop=(t == CCH - 1),
                )
            # per-group a = rstd, b = -mean * rstd
            gab = pp.tile([P, 2], F32, tag="gab")
            nc.vector.memset(gab[:], 0.0)
            gmean = wp.tile([P, 1], F32, tag="gmean")
            gtmp = wp.tile([P, 1], F32, tag="gtmp")
            nc.vector.tensor_scalar_mul(gmean[:GROUPS], gagg_ps[:GROUPS, 0:1],
                                        1.0 / GSZ)
            nc.vector.tensor_scalar_mul(gtmp[:GROUPS], gagg_ps[:GROUPS, 1:2],
                                        1.0 / GSZ)
            # gtmp := mean^2 - E[x^2] = -var
            nc.vector.scalar_tensor_tensor(
                out=gtmp[:GROUPS],
                in0=gmean[:GROUPS],
                scalar=gmean[:GROUPS],
                in1=gtmp[:GROUPS],
                op0=ALU.mult,
                op1=ALU.subtract,
            )
            # std = sqrt(-1 * gtmp + eps)
            epsb = wp.tile([P, 1], F32, tag="epsb")
            nc.vector.memset(epsb[:], EPS)
            nc.scalar.activation(gtmp[:GROUPS], gtmp[:GROUPS], AF.Sqrt,
                                 bias=epsb[:GROUPS], scale=-1.0)
            # dummy exp: pull the Exp ACT-table load off the critical path
            # (the first real exp otherwise eats a 1.3us table swap)
            dume = wp.tile([P, 1], F32, tag="dume")
            nc.scalar.activation(dume[:1], epsb[:1], AF.Exp, scale=1.0)
            nc.vector.reciprocal(gab[:GROUPS, 0:1], gtmp[:GROUPS])  # a = rstd
            nc.vector.tensor_mul(gtmp[:GROUPS], gmean[:GROUPS], gab[:GROUPS, 0:1])
            nc.vector.tensor_scalar_mul(gab[:GROUPS, 1:2], gtmp[:GROUPS], -1.0)

            # broadcast (a, b) to channels; fold GN into the fp8 weights:
            #   m_c = rstd_g * gamma_c ; a_c = (-mean*rstd)*gamma_c + beta_c
            #   w_eff[c, o] = qkvw[c, o] * m_c        (qkvw = 32*W^T)
            #   a2_c = 32 * a_c / m_c   (fp8; a-term via w_eff @ a2 / 1024)
            w_eff = pp.tile([P, CCH, 3 * C], FP8, tag="w_eff")
            a_col = pp.tile([P, CCH, 1], FP8, tag="a_col")
            chms = []
            for t in range(CCH):
                chab_ps = psp.tile([P, QT], F32, tag="p", name=f"chab_ps{t}")
                nc.tensor.matmul(chab_ps[:, :2], gnind2[:, t], gab[:],
                                 start=True, stop=True)
                chm = pp.tile([P, 1], F32, tag=f"chm{t}", name=f"chm{t}")
                cha = pp.tile([P, 1], F32, tag=f"cha{t}", name=f"cha{t}")
                nc.vector.tensor_mul(chm[:], chab_ps[:, 0:1], gnsc[:, t, None])
                nc.vector.scalar_tensor_tensor(
                    out=cha[:],
                    in0=chab_ps[:, 1:2],
                    scalar=gnsc[:, t, None],
                    in1=gnbi[:, t, None],
                    op0=ALU.mult,
                    op1=ALU.add,
                )
                chms.append(chm)
                nc.vector.tensor_scalar_mul(w_eff[:, t], qkvw[:, t], chm[:])
                # a2 = 32 * cha / chm  (fp8)
                rchm = wp.tile([P, 1], F32, tag=f"rchm{t}", name=f"rchm{t}")
                nc.vector.reciprocal(rchm[:], chm[:])
                nc.vector.scalar_tensor_tensor(
                    out=a_col[:, t],
                    in0=cha[:],
                    scalar=WS,
                    in1=rchm[:],
                    op0=ALU.mult,
                    op1=ALU.mult,
                )

            # ---- bias vectors: Wa terms via tiny matmuls ----
            # Q/K: qkb_eff[d, j] = qkb[d, j] + (W@a)[j*128+d] / 1024
            qkb_eff = pp.tile([P, 4], F32, tag="qkb_eff")
            for j in range(4):
                wa_ps = psp.tile([P, QT], F32, tag="p", name=f"wa_ps{j}")
                for t in range(CCH):
                    nc.tensor.matmul(
                        wa_ps[:, 0:1],
                        w_eff[:, t, j * P:(j + 1) * P],
                        a_col[:, t],
                        start=(t == 0),
                        stop=(t == CCH - 1),
                    )
                nc.vector.scalar_tensor_tensor(
                    out=qkb_eff[:, j, None],
                    in0=wa_ps[:, 0:1],
                    scalar=1.0 / (WS * WS),
                    in1=qkb[:, j, None],
                    op0=ALU.mult,
                    op1=ALU.add,
                )
            # V: vb_eff[*, d] = vbias[d] + (Wv@a)[d] / 1024, broadcast to all
            # partitions via a ones-column fp32 matmul.
            vr_ps = psp.tile([P, QT], F32, tag="p", name="vr_ps")
            for t in range(CCH):
                nc.tensor.matmul(
                    vr_ps[:1, :C],
                    a_col[:, t],
                    w_eff[:, t, 2 * C:3 * C],
                    start=(t == 0),
                    stop=(t == CCH - 1),
                )
            vrow = pp.tile([1, C], F32, tag="vrow")
            nc.vector.tensor_copy(vrow[:], vr_ps[:1, :C])
            vb_ps = psp.tile([P, QT], F32, tag="p", name="vb_ps")
            nc.tensor.matmul(vb_ps[:, :C], onescol[:], vrow[:],
                             start=True, stop=True)
            vb_eff = pp.tile([P, C], F32, tag="vb_eff")
            nc.vector.scalar_tensor_tensor(
                out=vb_eff[:],
                in0=vb_ps[:, :C],
                scalar=1.0 / (WS * WS),
                in1=vbias[:],
                op0=ALU.mult,
                op1=ALU.add,
            )

            # ---------------- QKV ----------------
            # Q, K in (d, n) fp8; V token-major fp8. All matmuls fp8 DR
            # (K=256 contraction in one pass). Q and K live in per-512-token
            # tiles (q_tiles[h] / k_tiles[h], planes = channel chunks) so a
            # late in-loop copy of block h never creates a (false, whole-
            # tile) dependency against concurrent score reads of other
            # blocks. Early-needed copies go on ACT (idle pre-attention),
            # the rest on DVE (idle during attention).
            q_tiles = [pp.tile([P, 2, QT], FP8, tag=f"qb{h}", name=f"qb{h}")
                       for h in range(NQ)]
            k_tiles = [pp.tile([P, 2, QT], FP8, tag=f"kb{h}", name=f"kb{h}")
                       for h in range(NQ)]
            v_sb = pp.tile([P, NKB, C], FP8, tag="v_sb")

            import itertools
            _tag_cycle = itertools.cycle([("s", 2), ("o", 1), ("s", 2), ("p", 1)])

            def emit_qk_half(j, h, engine, tag=None):
                # one [P, 512] half-block of Q (j<2) or K (j>=2); j%2 is the
                # channel chunk (plane) within the per-h tile.
                ns = slice(h * QT, (h + 1) * QT)
                dst = (q_tiles if j < 2 else k_tiles)[h][:, j % 2]
                tg, bf = tag if tag else ("s", 2)
                qp = psp.tile([P, QT], F32, tag=tg, bufs=bf,
                              name=f"qp{j}_{h}")
                nc.tensor.matmul(
                    qp[:],
                    w_eff[:, :, j * P:(j + 1) * P],
                    x_f8[:, :, ns],
                    start=True,
                    stop=True,
                    perf_mode=DR,
                )
                if engine == "act":
                    nc.scalar.activation(
                        dst, qp[:],
                        AF.Identity,
                        bias=qkb_eff[:, j, None],
                        scale=INV_WS,
                    )
                else:
                    nc.vector.tensor_scalar(
                        out=dst,
                        in0=qp[:],
                        scalar1=INV_WS,
                        scalar2=qkb_eff[:, j, None],
                        op0=ALU.mult,
                        op1=ALU.add,
                    )

            def emit_v_block(tp, engine="dve", tag=None):
                # two 128-token blocks of V: out [tok, 2, C]
                tg, bf = tag if tag else ("s", 2)
                vp = psp.tile([P, 2, C], F32, tag=tg, bufs=bf, name=f"vp{tp}")
                for k2 in range(2):
                    tb = 2 * tp + k2
                    nc.tensor.matmul(
                        vp[:, k2],
                        x_f8[:, :, tb * P:(tb + 1) * P],
                        w_eff[:, :, 2 * C:3 * C],
                        start=True,
                        stop=True,
                        perf_mode=DR,
                    )
                nc.vector.scalar_tensor_tensor(
                    out=v_sb[:, 2 * tp:2 * tp + 2],
                    in0=vp[:],
                    scalar=INV_WS,
                    in1=vb_eff[:, None, :].to_broadcast([P, 2, C]),
                    op0=ALU.mult,
                    op1=ALU.add,
                )

            # Upfront: K halves 0-3 + Q half 0 (copies on ACT, idle before
            # the first exp) and all of V (copies on DVE); PSUM tiles cycle
            # over every tag so no single bank chain serializes.
            for j in (2, 3, 0, 1):
                emit_qk_half(j, 0, "act", next(_tag_cycle))
            for h in (1, 2, 3):
                for j in (2, 3):
                    emit_qk_half(j, h, "act", next(_tag_cycle))
            for tp in range(NKB // 2):
                emit_v_block(tp, "dve", next(_tag_cycle))

            # K halves 4-7 and Q halves 1-7 are deadline-scheduled into the
            # attention loop (K half h feeds steps 2h..2h+1 of every q tile;
            # Q half h is first read at step 16h); copies on the idle DVE.
            pending = {}

            def sched(step, fn):
                pending.setdefault(step, []).append(fn)

            for h in (4, 5, 6, 7):
                for j in (2, 3):
                    sched(2 * (h - 4) + (j - 2),
                          lambda j=j, h=h: emit_qk_half(j, h, "dve"))
            for h in range(1, 8):
                for j in (0, 1):
                    sched(16 * h - 5 + j,
                          lambda j=j, h=h: emit_qk_half(j, h, "dve"))

            # ---------------- attention ----------------
            # global steps g = qt*NKP + i ; per step:
            #   scores(g):  2 DR matmuls -> s_ps [P, 2(kb), 512]
            #   exp(g):     1 ACT instr [P, 1024] -> pt fp8
            #   pv_den(g-2): 2 PV DR matmuls + 1 ones-DR matmul into
            #                o tile [P, 3, 512] (ch0, ch1, den)
            # PV runs TWO steps behind scores so the PE never waits on the
            # scores->exp->pt chain (exp latency > PE slack per step).
            # finalize is split: rec/obs (DVE) emit at (qt, 2) BEFORE
            # pv_den(qt, 0) so the o-tile reuse is ordered; proj/res emit
            # at (qt, 3).
            o_tiles = {}
            pt_tiles = {}

            def emit_scores_exp(g):
                qt, i = divmod(g, NKP)
                s_ps = psp.tile([P, 2, QT], F32, tag="s", bufs=2,
                                name=f"s_{g}")
                for k2 in range(2):
                    kb = 2 * i + k2
                    nc.tensor.matmul(
                        s_ps[:, k2],
                        k_tiles[kb // 4][:, :, (kb % 4) * P:(kb % 4 + 1) * P],
                        q_tiles[qt][:],
                        start=True,
                        stop=True,
                        perf_mode=DR,
                    )
                pt = wp.tile([P, 2, QT], FP8, tag="pt", bufs=6, name=f"pt_{g}")
                nc.scalar.activation(
                    pt.rearrange("p a b -> p (a b)"),
                    s_ps.rearrange("p a b -> p (a b)"),
                    AF.Exp, scale=ATT_SCALE)
                pt_tiles[g] = pt

            def emit_pv(g):
                qt, i = divmod(g, NKP)
                if i == 0:
                    o_tiles[qt] = psp.tile([P, 3, QT], F32, tag="o",
                                           name=f"o_{qt}")
                o = o_tiles[qt]
                pt = pt_tiles[g]
                for ch in range(2):
                    nc.tensor.matmul(
                        o[:, ch],
                        v_sb[:, 2 * i:2 * i + 2, ch * P:(ch + 1) * P],
                        pt[:],
                        start=(i == 0),
                        stop=(i == NKP - 1),
                        perf_mode=DR,
                    )

            def emit_den(g):
                # denominator rides one step behind PV: its i==0 matmul must
                # wait for rec(qt-1) to read the previous den bank, and that
                # chain resolves ~one step later than the obs copies.
                qt, i = divmod(g, NKP)
                o = o_tiles[qt]
                pt = pt_tiles.pop(g)
                nc.tensor.matmul(
                    o[:, 2],
                    ones_f8[:],
                    pt[:],
                    start=(i == 0),
                    stop=(i == NKP - 1),
                    perf_mode=DR,
                )

            def finalize_a(qt):
                # UN-normalized bf16 copies of the PV output (frees the o
                # tile without waiting on the reciprocal chain) + the
                # denominator reciprocal. Normalization happens after proj:
                # proj is linear, so proj(out/den) == proj(out)/den.
                o = o_tiles.pop(qt)
                obs = wp.tile([P, 2, QT], BF16, tag="obs", bufs=2,
                              name=f"obs{qt}")
                nc.vector.tensor_copy(obs[:, 0], o[:, 0])
                nc.vector.tensor_copy(obs[:, 1], o[:, 1])
                rec = wp.tile([P, QT], F32, tag="rec", bufs=2, name=f"rec{qt}")
                nc.vector.reciprocal_approx_fast(rec[:], o[:, 2])
                return obs, rec

            def finalize_b(qt, obs, rec, t):
                # one output-channel-chunk of proj + residual + store
                qs = slice(qt * QT, (qt + 1) * QT)
                p_ps = psp.tile([P, QT], F32, tag="p", name=f"pp_{qt}_{t}")
                nc.tensor.matmul(p_ps[:],
                                 projw[:, 0, t * P:(t + 1) * P],
                                 obs[:, 0], start=True, stop=False)
                nc.tensor.matmul(p_ps[:],
                                 projw[:, 1, t * P:(t + 1) * P],
                                 obs[:, 1], start=False, stop=True)
                tmp = wp.tile([P, QT], F32, tag="tmp", bufs=2,
                              name=f"tmp{qt}_{t}")
                nc.vector.tensor_mul(tmp[:], p_ps[:], rec[:])
                res = wp.tile([P, QT], F32, tag="res", bufs=4,
                              name=f"res{qt}_{t}")
                nc.vector.scalar_tensor_tensor(
                    out=res[:],
                    in0=tmp[:],
                    scalar=projb[:, t, None],
                    in1=x_sb[:, t, qs],
                    op0=ALU.add,
                    op1=ALU.add,
                )
                nc.sync.dma_start(out_d[t * P:(t + 1) * P, qs], res[:])

            NST = NQ * NKP
            obs_pending = None
            for g in range(NST):
                qt, i = divmod(g, NKP)
                emit_scores_exp(g)
                for fn in pending.pop(g, ()):
                    fn()
                if g > 2:
                    emit_den(g - 3)      # includes den(qt-1, 15) at i == 2
                if qt > 0 and i == 2:
                    obs_pending = (qt - 1,) + finalize_a(qt - 1)
                if g > 1:
                    emit_pv(g - 2)       # PV(qt, 0) waits only on the obs
                if qt > 0 and i == 5:
                    finalize_b(*obs_pending, t=0)
                if qt > 0 and i == 9:
                    finalize_b(*obs_pending, t=1)
                    obs_pending = None
            emit_pv(NST - 2)
            emit_den(NST - 3)
            emit_pv(NST - 1)
            emit_den(NST - 2)
            emit_den(NST - 1)
            oq, obs_l, rec_l = (NQ - 1,) + finalize_a(NQ - 1)
            finalize_b(oq, obs_l, rec_l, t=0)
            finalize_b(oq, obs_l, rec_l, t=1)

    nc.finalize()
    return nc


# ---------------------------------------------------------------------------
# host side
# ---------------------------------------------------------------------------

def _prep_core_inputs(inputs, n_tok=H * W):
    """Build the per-core in_maps (shared weight tensors + per-core x)."""
    CCH = C // P
    f32 = np.float32
    bf16 = ml_dtypes.bfloat16
    fp8 = mybir.dt.np(FP8)

    x = np.asarray(inputs["x"], f32).reshape(B, C, n_tok)
    gn_scale = np.asarray(inputs["gn_scale"], f32)
    gn_bias = np.asarray(inputs["gn_bias"], f32)
    qkv_w = np.asarray(inputs["qkv_w"], f32)
    qkv_b = np.asarray(inputs["qkv_b"], f32)
    proj_w = np.asarray(inputs["proj_w"], f32)
    proj_b = np.asarray(inputs["proj_b"], f32)

    qkv_wt = (np.ascontiguousarray(qkv_w.T) * WS).reshape(CCH, P, 3 * C).astype(fp8)
    v_bias = qkv_b[2 * C:].astype(f32)
    proj_wt = np.ascontiguousarray(proj_w.T).reshape(CCH, P, C).astype(bf16)

    # packed per-partition scalars: qkb(4), projb(2), gnsc(2), gnbi(2)
    smalls = np.concatenate([
        qkv_b[:2 * C].reshape(4, P),
        proj_b.reshape(CCH, P),
        gn_scale.reshape(CCH, P),
        gn_bias.reshape(CCH, P),
    ], axis=0).astype(f32)

    ch = np.arange(C)
    gn_ind = np.zeros((CCH, P, P), f32)
    gn_ind[ch // P, ch % P, ch // (C // GROUPS)] = 1.0
    gn_ind2 = np.zeros((CCH, P, P), f32)
    for t in range(CCH):
        gn_ind2[t, :GROUPS, :] = gn_ind[t, :, :GROUPS].T
    gn_inds = np.stack([gn_ind, gn_ind2])

    shared = {
        "qkv_wt": qkv_wt,
        "v_bias": v_bias,
        "proj_wt": proj_wt,
        "smalls": smalls,
        "gn_inds": gn_inds,
    }
    x_f8 = x.reshape(B, CCH, P, n_tok).astype(fp8)
    return [
        dict(shared, x=np.ascontiguousarray(x[i]),
             x_f8=np.ascontiguousarray(x_f8[i]))
        for i in range(B)
    ]


_NC_CACHE = {}
LAST_RESULT = None  # BassKernelResults of the most recent run (for test.py)


def _get_nc():
    if "nc" not in _NC_CACHE:
        _NC_CACHE["nc"] = build_nc()
    return _NC_CACHE["nc"]


def kernel(**inputs) -> np.ndarray:
    global LAST_RESULT
    from concourse.bass_utils import run_bass_kernel_spmd

    nc = _get_nc()
    in_maps = _prep_core_inputs(inputs)
    res = run_bass_kernel_spmd(nc, in_maps, list(range(N_CORES)))
    LAST_RESULT = res
    out = np.stack([np.asarray(res.results[i]["out"]) for i in range(B)])
    return out.reshape(B, C, H, W).astype(np.float32)


# revision 50
# speedup vs baseline: 1.0504x; 1.0393x over previous
"""Trainium2 Bass kernel for nn_AttentionBlock (GroupNorm + single-head
self-attention + proj + residual), data-parallel over batch on 8 cores.

Contract: kernel(**inputs) takes the FULL unsharded inputs
  x (8, 256, 64, 64) f32, gn_scale (256,), gn_bias (256,),
  qkv_w (768, 256), qkv_b (768,), proj_w (256, 256), proj_b (256,)
and returns the FULL output (8, 256, 64, 64) f32.

v2 design (from the v1 NTFF trace: PE 90% busy on matmuls, ACT co-bound
on exp, DVE saturated by denominator accumulation):
  - GroupNorm folded into the QKV weights: w_eff[c,o] = 32*W[o,c]*m_c on
    device (m_c = rstd*gamma per channel), so no xn tensor is ever
    materialized. The additive GN term (a_c) becomes per-output biases
    via tiny matmuls (W@a). x is shipped from host in BOTH f32 (GN stats
    + residual) and fp8 (QKV matmul operand).
  - QKV/scores/PV all fp8 DoubleRow (K=256 in one pass).
  - Softmax denominator on the PE: a ones-lhsT DR matmul per key-block
    pair accumulates den[q] into the same PSUM tile group as the PV
    output (tile [P, 3, 512]: ch0, ch1, den) -> zero DVE work in the
    steady loop.
  - Steady state per 512-q-tile step: PE 5 matmuls (2 scores, 2 PV,
    1 den) ~1.1us; ACT one 1024-wide exp ~1.1us. PV/den run one step
    behind scores so ACT never waits on PE.
  - PSUM banks: scores 2x[P,2,512]=4, out+den [P,3,512]=3, proj 1 = 8.
"""

import os
import sys

import numpy as np

for _p in (
    "/opt/trn_rl_repo",
    "/root/.axon_site",
    "/root/.axon_site/_ro/trn_rl_repo",
    "/root/.axon_site/_ro/pypackages",
):
    if os.path.isdir(_p) and _p not in sys.path:
        sys.path.append(_p)

import ml_dtypes  # noqa: E402

import concourse.bass as bass  # noqa: E402
import concourse.mybir as mybir  # noqa: E402
import concourse.tile as tile  # noqa: E402
from concourse import bacc  # noqa: E402

F32 = mybir.dt.float32
BF16 = mybir.dt.bfloat16
FP8 = mybir.dt.float8e4
AF = mybir.ActivationFunctionType
ALU = mybir.AluOpType
DR = mybir.MatmulPerfMode.DoubleRow

B, C, H, W = 8, 256, 64, 64
GROUPS = 8
EPS = 1e-5
P = 128
N_CORES = 8
ATT_SCALE = float(C) ** -0.5  # 1/16
WS = 32.0                     # host pre-scale on fp8 qkv weights
INV_WS = 1.0 / WS


def build_nc(n_tok=H * W):
    """Build the single-core Bass program (SPMD across 8 cores)."""
    CCH = C // P            # channel chunks (2)
    QT = 512                # q-tile width (one PSUM bank of f32)
    NQ = n_tok // QT        # number of q tiles (8)
    NKB = n_tok // P        # number of 128-token key blocks (32)
    NKP = NKB // 2          # key-block pairs per q tile (16)
    GSZ = C // GROUPS       # channels per group (32)

    nc = bacc.Bacc()

    # ---- DRAM I/O (per-core tensors; host shards batch over cores) ----
    x_d = nc.dram_tensor("x", [C, n_tok], F32, kind="ExternalInput")
    xf8_d = nc.dram_tensor("x_f8", [CCH, P, n_tok], FP8, kind="ExternalInput")
    qkvw_d = nc.dram_tensor("qkv_wt", [CCH, P, 3 * C], FP8, kind="ExternalInput")
    projw_d = nc.dram_tensor("proj_wt", [CCH, P, C], FP8, kind="ExternalInput")
    # packed per-partition scalars: qkb(4), projb(2), gnsc(2), gnbi(2), vb(2)
    smalls_d = nc.dram_tensor("smalls", [12, P], F32, kind="ExternalInput")
    # packed indicators: [0] = gn_ind (c -> group), [1] = gn_ind2 (group -> c)
    inds_d = nc.dram_tensor("gn_inds", [2, CCH, P, P], F32, kind="ExternalInput")
    out_d = nc.dram_tensor("out", [C, n_tok], F32, kind="ExternalOutput")

    with tile.TileContext(nc) as tc:
        with (
            tc.tile_pool(name="persist", bufs=1) as pp,
            tc.tile_pool(name="work", bufs=3) as wp,
            tc.tile_pool(name="ps", bufs=1, space="PSUM") as psp,
        ):
            # ---------------- DMAs: x_f8 first (stats + QKV input) --------
            x_f8 = pp.tile([P, CCH, n_tok], FP8, tag="x_f8")
            XPC = 4
            for pc in range(XPC):
                xs = slice(pc * (n_tok // XPC), (pc + 1) * (n_tok // XPC))
                nc.sync.dma_start(
                    x_f8[:, :, xs],
                    xf8_d.rearrange("t p n -> p t n")[:, :, xs],
                )
            qkvw = pp.tile([P, CCH, 3 * C], FP8, tag="qkvw")
            nc.sync.dma_start(qkvw[:], qkvw_d.rearrange("t p o -> p t o"))
            smalls = pp.tile([P, 12], F32, tag="smalls")
            nc.sync.dma_start(smalls[:], smalls_d.rearrange("j p -> p j"))
            qkb = smalls[:, 0:4]
            projb = smalls[:, 4:6]
            gnsc = smalls[:, 6:8]
            gnbi = smalls[:, 8:10]
            vbias = smalls[:, 10:12]        # per d-chunk [P, 1] columns
            inds = pp.tile([P, 2, CCH, P], F32, tag="inds")
            nc.sync.dma_start(inds[:], inds_d.rearrange("w t p g -> p w t g"))
            gnind2 = inds[:, 1]
            projw = pp.tile([P, CCH, C], FP8, tag="projw")
            nc.sync.dma_start(projw[:], projw_d.rearrange("t p o -> p t o"))
            # 32.0 in the den lhsT folds the 1/32 proj-weight prescale into
            # the softmax reciprocal (rec = 1/(32*den))
            ones_f8 = pp.tile([P, 2, P], FP8, tag="ones_f8")
            nc.vector.memset(ones_f8[:], 32.0)
            # vbias/32 per chunk; the finalize-time V-shift constants
            # ((Wv@a + bv)/32) land in bvs once the GN fold is known
            bvs0 = pp.tile([P, 2], F32, tag="bvs0")
            nc.vector.tensor_scalar_mul(bvs0[:], vbias[:], INV_WS)
            bvs = pp.tile([P, 2], F32, tag="bvs")

            # ---------------- GN stats (bn_stats on the fp8 x) ------------
            # quantization noise on mean/var of 128k samples is ~1e-4
            # relative -- irrelevant against the 2e-2 budget.
            stats = pp.tile([P, CCH, 2], F32, tag="stats")
            for t in range(CCH):
                bn6 = wp.tile([P, n_tok // 512, 6], F32, tag="bn6")
                xv = x_f8[:, t].rearrange("p (a b) -> p a b", b=512)
                for a in range(n_tok // 512):
                    nc.vector.bn_stats(bn6[:, a], xv[:, a])
                nc.vector.bn_aggr(stats[:, t], bn6[:])
                # stats col1 := mean^2 + var = E[x^2] (col0 stays mean)
                nc.vector.scalar_tensor_tensor(
                    out=stats[:, t, 1:2],
                    in0=stats[:, t, 0:1],
                    scalar=stats[:, t, 0:1],
                    in1=stats[:, t, 1:2],
                    op0=ALU.mult,
                    op1=ALU.add,
                )
            # f32 x for the residual: queued last, consumed from finalize()
            # well into the attention phase.
            x_sb = pp.tile([P, CCH, n_tok], F32, tag="x_sb")
            for t in range(CCH):
                for pc in range(XPC):
                    xs = slice(pc * (n_tok // XPC), (pc + 1) * (n_tok // XPC))
                    nc.sync.dma_start(x_sb[:, t, xs], x_d[t * P:(t + 1) * P, xs])

            # group aggregation: gagg[g, j] = sum_{c in group g} stats[c, j]
            gagg_ps = psp.tile([P, QT], F32, tag="p", name="gagg_ps")
            for t in range(CCH):
                nc.tensor.matmul(
                    gagg_ps[:, :2],
                    inds[:, 0, t],
                    stats[:, t],
                    start=(t == 0),
                    stop=(t == CCH - 1),
                )
            # per-group a = rstd, b = -mean * rstd. gn_ind is host-scaled by
            # 1/GSZ, so gagg cols are already (mean, E[x^2]) per group.
            gab = pp.tile([P, 2], F32, tag="gab")
            nc.vector.memset(gab[:], 0.0)
            gsb = wp.tile([P, 2], F32, tag="gsb")
            nc.vector.tensor_copy(gsb[:GROUPS], gagg_ps[:GROUPS, :2])
            gtmp = wp.tile([P, 1], F32, tag="gtmp")
            # gtmp := mean^2 - E[x^2] = -var
            nc.vector.scalar_tensor_tensor(
                out=gtmp[:GROUPS],
                in0=gsb[:GROUPS, 0:1],
                scalar=gsb[:GROUPS, 0:1],
                in1=gsb[:GROUPS, 1:2],
                op0=ALU.mult,
                op1=ALU.subtract,
            )
            # std = sqrt(-1 * gtmp + eps)
            epsb = wp.tile([P, 1], F32, tag="epsb")
            nc.vector.memset(epsb[:], EPS)
            nc.scalar.activation(gtmp[:GROUPS], gtmp[:GROUPS], AF.Sqrt,
                                 bias=epsb[:GROUPS], scale=-1.0)
            # dummy exp AFTER the sqrt (data-dependent so the scheduler
            # cannot hoist it): loads the Exp ACT table once, and both the
            # Identity copies and the real exps then run out of that table.
            dume = wp.tile([P, 1], F32, tag="dume")
            nc.scalar.activation(dume[:1], gtmp[:1], AF.Exp, scale=1.0)
            nc.vector.reciprocal(gab[:GROUPS, 0:1], gtmp[:GROUPS])  # a = rstd
            nc.vector.tensor_mul(gtmp[:GROUPS], gsb[:GROUPS, 0:1],
                                 gab[:GROUPS, 0:1])
            nc.vector.tensor_scalar_mul(gab[:GROUPS, 1:2], gtmp[:GROUPS], -1.0)

            # broadcast (a, b) to channels; fold GN into the fp8 weights:
            #   m_c = rstd_g * gamma_c ; a_c = (-mean*rstd)*gamma_c + beta_c
            #   w_eff[c, o] = qkvw[c, o] * m_c        (qkvw = 32*W^T)
            #   a2_c = 32 * a_c / m_c   (fp8; a-term via w_eff @ a2 / 1024)
            w_eff = pp.tile([P, CCH, 3 * C], FP8, tag="w_eff")
            a_col = pp.tile([P, CCH, 1], FP8, tag="a_col")
            chms = []
            for t in range(CCH):
                chab_ps = psp.tile([P, QT], F32, tag="p", name=f"chab_ps{t}")
                nc.tensor.matmul(chab_ps[:, :2], gnind2[:, t], gab[:],
                                 start=True, stop=True)
                chm = pp.tile([P, 1], F32, tag=f"chm{t}", name=f"chm{t}")
                cha = pp.tile([P, 1], F32, tag=f"cha{t}", name=f"cha{t}")
                nc.vector.tensor_mul(chm[:], chab_ps[:, 0:1], gnsc[:, t, None])
                nc.vector.scalar_tensor_tensor(
                    out=cha[:],
                    in0=chab_ps[:, 1:2],
                    scalar=gnsc[:, t, None],
                    in1=gnbi[:, t, None],
                    op0=ALU.mult,
                    op1=ALU.add,
                )
                chms.append(chm)
                nc.vector.tensor_scalar_mul(w_eff[:, t], qkvw[:, t], chm[:])
                # a2 = 32 * a_c (fp8); the Wa bias matmuls use the RAW qkvw
                # (32*W), so W@a = qkvw^T @ a2 / 1024 -- no w_eff dependency
                nc.vector.tensor_scalar_mul(a_col[:, t], cha[:], WS)

            # ---- additive-GN bias vectors via tiny matmuls on raw qkvw ----
            # j<4: qkb_eff = qkb + (W@a)      (Q/K, applied at the copies)
            # j>=4: bvs = (Wv@a + bv)/32      (V, applied at finalize via
            #                                  the den row: obs = o + c*den)
            qkb_eff = pp.tile([P, 4], F32, tag="qkb_eff")
            for j in range(6):
                wa_ps = psp.tile([P, QT], F32, tag="p", name=f"wa_ps{j}")
                for t in range(CCH):
                    nc.tensor.matmul(
                        wa_ps[:, 0:1],
                        qkvw[:, t, j * P:(j + 1) * P],
                        a_col[:, t],
                        start=(t == 0),
                        stop=(t == CCH - 1),
                    )
                if j < 4:
                    nc.vector.scalar_tensor_tensor(
                        out=qkb_eff[:, j, None],
                        in0=wa_ps[:, 0:1],
                        scalar=1.0 / (WS * WS),
                        in1=qkb[:, j, None],
                        op0=ALU.mult,
                        op1=ALU.add,
                    )
                else:
                    nc.vector.scalar_tensor_tensor(
                        out=bvs[:, j - 4, None],
                        in0=wa_ps[:, 0:1],
                        scalar=1.0 / (WS * WS * WS),
                        in1=bvs0[:, j - 4, None],
                        op0=ALU.mult,
                        op1=ALU.add,
                    )

            # ---------------- QKV ----------------
            # Q, K in (d, n) fp8; V token-major fp8. All matmuls fp8 DR
            # (K=256 contraction in one pass). Q and K live in per-512-token
            # tiles (q_tiles[h] / k_tiles[h], planes = channel chunks) so a
            # late in-loop copy of block h never creates a (false, whole-
            # tile) dependency against concurrent score reads of other
            # blocks. Early-needed copies go on ACT (idle pre-attention),
            # the rest on DVE (idle during attention).
            q_tiles = [pp.tile([P, 2, QT], FP8, tag=f"qb{h}", name=f"qb{h}")
                       for h in range(NQ)]
            k_tiles = [pp.tile([P, 2, QT], FP8, tag=f"kb{h}", name=f"kb{h}")
                       for h in range(NQ)]
            v_sb = pp.tile([P, NKB, C], FP8, tag="v_sb")

            def emit_qk_half(j, h, engine, tag=None):
                # one [P, 512] half-block of Q (j<2) or K (j>=2); j%2 is the
                # channel chunk (plane) within the per-h tile. In-loop
                # halves use the "p" bank (never the "s" rotation, which
                # would delay scores two allocations later).
                ns = slice(h * QT, (h + 1) * QT)
                dst = (q_tiles if j < 2 else k_tiles)[h][:, j % 2]
                tg, bf = tag if tag else ("p", 1)
                qp = psp.tile([P, QT], F32, tag=tg, bufs=bf,
                              name=f"qp{j}_{h}")
                nc.tensor.matmul(
                    qp[:],
                    w_eff[:, :, j * P:(j + 1) * P],
                    x_f8[:, :, ns],
                    start=True,
                    stop=True,
                    perf_mode=DR,
                )
                if engine == "act":
                    nc.scalar.activation(
                        dst, qp[:],
                        AF.Identity,
                        bias=qkb_eff[:, j, None],
                        scale=INV_WS,
                    )
                else:
                    nc.vector.tensor_scalar(
                        out=dst,
                        in0=qp[:],
                        scalar1=INV_WS,
                        scalar2=qkb_eff[:, j, None],
                        op0=ALU.mult,
                        op1=ALU.add,
                    )

            def emit_v_block(tp, engine="dve"):
                # two 128-token blocks of V: out [tok, 2, C]. The GN/bias
                # shift is recovered at finalize, so this is a pure scaled
                # copy and can run on either ACT or DVE.
                vp = psp.tile([P, 2, C], F32, tag="s", bufs=2, name=f"vp{tp}")
                for k2 in range(2):
                    tb = 2 * tp + k2
                    nc.tensor.matmul(
                        vp[:, k2],
                        x_f8[:, :, tb * P:(tb + 1) * P],
                        w_eff[:, :, 2 * C:3 * C],
                        start=True,
                        stop=True,
                        perf_mode=DR,
                    )
                if engine == "act":
                    nc.scalar.activation(
                        v_sb[:, 2 * tp:2 * tp + 2],
                        vp[:], AF.Identity, bias=0.0, scale=INV_WS)
                else:
                    nc.vector.tensor_scalar_mul(
                        v_sb[:, 2 * tp:2 * tp + 2], vp[:], INV_WS)

            # Upfront: Q/K on their own two psum banks (o/p alternating,
            # ACT copies); V on the two "s" slots (copies alternate DVE/ACT)
            # -- two decoupled MM->copy streams.
            for r, (j, h) in enumerate([(2, 0), (3, 0), (0, 0), (1, 0),
                                        (2, 1), (3, 1), (2, 2), (3, 2),
                                        (2, 3), (3, 3)]):
                emit_qk_half(j, h, "act", ("o", 1) if r % 2 else ("p", 1))
            for tp in range(NKB // 2):
                emit_v_block(tp, "dve" if tp % 2 else "act")

            # K halves 4-7 and Q halves 1-7 are deadline-scheduled into the
            # attention loop (K half h feeds steps 2h..2h+1 of every q tile;
            # Q half h is first read at step 16h); copies on the idle DVE.
            pending = {}

            def sched(step, fn):
                pending.setdefault(step, []).append(fn)

            for h in (4, 5, 6, 7):
                for j in (2, 3):
                    sched(2 * (h - 4) + (j - 2),
                          lambda j=j, h=h: emit_qk_half(j, h, "dve"))
            for h in range(1, 8):
                for j in (0, 1):
                    sched(16 * h - 5 + j,
                          lambda j=j, h=h: emit_qk_half(j, h, "dve"))

            # ---------------- attention ----------------
            # global steps g = qt*NKP + i ; per step:
            #   scores(g):  2 DR matmuls -> s_ps [P, 2(kb), 512]
            #   exp(g):     1 ACT instr [P, 1024] -> pt fp8
            #   pv_den(g-2): 2 PV DR matmuls + 1 ones-DR matmul into
            #                o tile [P, 3, 512] (ch0, ch1, den)
            # PV runs TWO steps behind scores so the PE never waits on the
            # scores->exp->pt chain (exp latency > PE slack per step).
            # finalize is split: rec/obs (DVE) emit at (qt, 2) BEFORE
            # pv_den(qt, 0) so the o-tile reuse is ordered; proj/res emit
            # at (qt, 3).
            o_tiles = {}
            pt_tiles = {}

            def emit_scores_exp(g):
                qt, i = divmod(g, NKP)
                s_ps = psp.tile([P, 2, QT], F32, tag="s", bufs=2,
                                name=f"s_{g}")
                for k2 in range(2):
                    kb = 2 * i + k2
                    nc.tensor.matmul(
                        s_ps[:, k2],
                        k_tiles[kb // 4][:, :, (kb % 4) * P:(kb % 4 + 1) * P],
                        q_tiles[qt][:],
                        start=True,
                        stop=True,
                        perf_mode=DR,
                    )
                pt = wp.tile([P, 2, QT], FP8, tag="pt", bufs=6, name=f"pt_{g}")
                nc.scalar.activation(
                    pt.rearrange("p a b -> p (a b)"),
                    s_ps.rearrange("p a b -> p (a b)"),
                    AF.Exp, scale=ATT_SCALE)
                pt_tiles[g] = pt

            def emit_pv(g):
                qt, i = divmod(g, NKP)
                if i == 0:
                    o_tiles[qt] = psp.tile([P, 3, QT], F32, tag="o",
                                           name=f"o_{qt}")
                o = o_tiles[qt]
                pt = pt_tiles[g]
                for ch in range(2):
                    nc.tensor.matmul(
                        o[:, ch],
                        v_sb[:, 2 * i:2 * i + 2, ch * P:(ch + 1) * P],
                        pt[:],
                        start=(i == 0),
                        stop=(i == NKP - 1),
                        perf_mode=DR,
                    )

            def emit_den(g):
                # denominator rides one step behind PV: its i==0 matmul must
                # wait for rec(qt-1) to read the previous den bank, and that
                # chain resolves ~one step later than the obs copies.
                qt, i = divmod(g, NKP)
                o = o_tiles[qt]
                pt = pt_tiles.pop(g)
                nc.tensor.matmul(
                    o[:, 2],
                    ones_f8[:],
                    pt[:],
                    start=(i == 0),
                    stop=(i == NKP - 1),
                    perf_mode=DR,
                )

            def finalize_a(qt):
                # UN-normalized fp8 copies of the PV output with the V-shift
                # constant re-applied through the den row
                # (obs = o + (c/32)*(32*den)); frees the o tile without
                # waiting on the reciprocal. Normalization happens after
                # proj: proj is linear, so proj(out/den) == proj(out)/den.
                o = o_tiles.pop(qt)
                den_sb = wp.tile([P, QT], F32, tag="densb", bufs=2,
                                 name=f"den{qt}")
                nc.vector.tensor_copy(den_sb[:], o[:, 2])
                obs = wp.tile([P, 2, QT], FP8, tag="obs", bufs=2,
                              name=f"obs{qt}")
                for ch in range(2):
                    nc.vector.scalar_tensor_tensor(
                        out=obs[:, ch],
                        in0=den_sb[:],
                        scalar=bvs[:, ch, None],
                        in1=o[:, ch],
                        op0=ALU.mult,
                        op1=ALU.add,
                    )
                rec = wp.tile([P, QT], F32, tag="rec", bufs=2, name=f"rec{qt}")
                nc.vector.reciprocal_approx_fast(rec[:], den_sb[:])
                return obs, rec

            def finalize_b(qt, obs, rec, t):
                # one output-channel-chunk of proj + residual + store;
                # proj is a single fp8 DoubleRow matmul (32x weights and the
                # 1/32 both folded into rec via the 32.0 den lhsT)
                qs = slice(qt * QT, (qt + 1) * QT)
                p_ps = psp.tile([P, QT], F32, tag="p", name=f"pp_{qt}_{t}")
                nc.tensor.matmul(p_ps[:],
                                 projw[:, :, t * P:(t + 1) * P],
                                 obs[:], start=True, stop=True,
                                 perf_mode=DR)
                tmp = wp.tile([P, QT], F32, tag="tmp", bufs=2,
                              name=f"tmp{qt}_{t}")
                nc.vector.tensor_mul(tmp[:], p_ps[:], rec[:])
                res = wp.tile([P, QT], F32, tag="res", bufs=4,
                              name=f"res{qt}_{t}")
                nc.vector.scalar_tensor_tensor(
                    out=res[:],
                    in0=tmp[:],
                    scalar=projb[:, t, None],
                    in1=x_sb[:, t, qs],
                    op0=ALU.add,
                    op1=ALU.add,
                )
                nc.sync.dma_start(out_d[t * P:(t + 1) * P, qs], res[:])

            NST = NQ * NKP
            obs_pending = None
            for g in range(NST):
                qt, i = divmod(g, NKP)
                emit_scores_exp(g)
                for fn in pending.pop(g, ()):
                    fn()
                if g > 2:
                    emit_den(g - 3)      # includes den(qt-1, 15) at i == 2
                if qt > 0 and i == 2:
                    obs_pending = (qt - 1,) + finalize_a(qt - 1)
                if g > 1:
                    emit_pv(g - 2)       # PV(qt, 0) waits only on the obs
                if qt > 0 and i == 5:
                    finalize_b(*obs_pending, t=0)
                if qt > 0 and i == 9:
                    finalize_b(*obs_pending, t=1)
                    obs_pending = None
            emit_pv(NST - 2)
            emit_den(NST - 3)
            emit_pv(NST - 1)
            emit_den(NST - 2)
            emit_den(NST - 1)
            oq, obs_l, rec_l = (NQ - 1,) + finalize_a(NQ - 1)
            finalize_b(oq, obs_l, rec_l, t=0)
            finalize_b(oq, obs_l, rec_l, t=1)

    nc.finalize()
    return nc


# ---------------------------------------------------------------------------
# host side
# ---------------------------------------------------------------------------

def _prep_core_inputs(inputs, n_tok=H * W):
    """Build the per-core in_maps (shared weight tensors + per-core x)."""
    CCH = C // P
    f32 = np.float32
    bf16 = ml_dtypes.bfloat16
    fp8 = mybir.dt.np(FP8)

    x = np.asarray(inputs["x"], f32).reshape(B, C, n_tok)
    gn_scale = np.asarray(inputs["gn_scale"], f32)
    gn_bias = np.asarray(inputs["gn_bias"], f32)
    qkv_w = np.asarray(inputs["qkv_w"], f32)
    qkv_b = np.asarray(inputs["qkv_b"], f32)
    proj_w = np.asarray(inputs["proj_w"], f32)
    proj_b = np.asarray(inputs["proj_b"], f32)

    qkv_wt = (np.ascontiguousarray(qkv_w.T) * WS).reshape(CCH, P, 3 * C).astype(fp8)
    proj_wt = (np.ascontiguousarray(proj_w.T) * WS).reshape(CCH, P, C).astype(fp8)

    # packed per-partition scalars: qkb(4), projb(2), gnsc(2), gnbi(2), vb(2)
    smalls = np.concatenate([
        qkv_b[:2 * C].reshape(4, P),
        proj_b.reshape(CCH, P),
        gn_scale.reshape(CCH, P),
        gn_bias.reshape(CCH, P),
        qkv_b[2 * C:].reshape(CCH, P),
    ], axis=0).astype(f32)

    ch = np.arange(C)
    gn_ind = np.zeros((CCH, P, P), f32)
    # pre-scaled by 1/GSZ: the group-aggregation matmul yields means
    gn_ind[ch // P, ch % P, ch // (C // GROUPS)] = 1.0 / (C // GROUPS)
    gn_ind2 = np.zeros((CCH, P, P), f32)
    for t in range(CCH):
        gn_ind2[t, :GROUPS, :] = (gn_ind[t, :, :GROUPS] > 0).T.astype(f32)
    gn_inds = np.stack([gn_ind, gn_ind2])

    shared = {
        "qkv_wt": qkv_wt,
        "proj_wt": proj_wt,
        "smalls": smalls,
        "gn_inds": gn_inds,
    }
    x_f8 = x.reshape(B, CCH, P, n_tok).astype(fp8)
    return [
        dict(shared, x=np.ascontiguousarray(x[i]),
             x_f8=np.ascontiguousarray(x_f8[i]))
        for i in range(B)
    ]


_NC_CACHE = {}
LAST_RESULT = None  # BassKernelResults of the most recent run (for test.py)


def _get_nc():
    if "nc" not in _NC_CACHE:
        _NC_CACHE["nc"] = build_nc()
    return _NC_CACHE["nc"]


def kernel(**inputs) -> np.ndarray:
    global LAST_RESULT
    from concourse.bass_utils import run_bass_kernel_spmd

    nc = _get_nc()
    in_maps = _prep_core_inputs(inputs)
    res = run_bass_kernel_spmd(nc, in_maps, list(range(N_CORES)))
    LAST_RESULT = res
    out = np.stack([np.asarray(res.results[i]["out"]) for i in range(B)])
    return out.reshape(B, C, H, W).astype(np.float32)


# revision 55
# speedup vs baseline: 1.0920x; 1.0396x over previous
"""Trainium2 Bass kernel for nn_AttentionBlock (GroupNorm + single-head
self-attention + proj + residual), data-parallel over batch on 8 cores.

Contract: kernel(**inputs) takes the FULL unsharded inputs
  x (8, 256, 64, 64) f32, gn_scale (256,), gn_bias (256,),
  qkv_w (768, 256), qkv_b (768,), proj_w (256, 256), proj_b (256,)
and returns the FULL output (8, 256, 64, 64) f32.

v2 design (from the v1 NTFF trace: PE 90% busy on matmuls, ACT co-bound
on exp, DVE saturated by denominator accumulation):
  - GroupNorm folded into the QKV weights: w_eff[c,o] = 32*W[o,c]*m_c on
    device (m_c = rstd*gamma per channel), so no xn tensor is ever
    materialized. The additive GN term (a_c) becomes per-output biases
    via tiny matmuls (W@a). x is shipped from host in BOTH f32 (GN stats
    + residual) and fp8 (QKV matmul operand).
  - QKV/scores/PV all fp8 DoubleRow (K=256 in one pass).
  - Softmax denominator on the PE: a ones-lhsT DR matmul per key-block
    pair accumulates den[q] into the same PSUM tile group as the PV
    output (tile [P, 3, 512]: ch0, ch1, den) -> zero DVE work in the
    steady loop.
  - Steady state per 512-q-tile step: PE 5 matmuls (2 scores, 2 PV,
    1 den) ~1.1us; ACT one 1024-wide exp ~1.1us. PV/den run one step
    behind scores so ACT never waits on PE.
  - PSUM banks: scores 2x[P,2,512]=4, out+den [P,3,512]=3, proj 1 = 8.
"""

import os
import sys

import numpy as np

for _p in (
    "/opt/trn_rl_repo",
    "/root/.axon_site",
    "/root/.axon_site/_ro/trn_rl_repo",
    "/root/.axon_site/_ro/pypackages",
):
    if os.path.isdir(_p) and _p not in sys.path:
        sys.path.append(_p)

import ml_dtypes  # noqa: E402

import concourse.bass as bass  # noqa: E402
import concourse.mybir as mybir  # noqa: E402
import concourse.tile as tile  # noqa: E402
from concourse import bacc  # noqa: E402

F32 = mybir.dt.float32
BF16 = mybir.dt.bfloat16
FP8 = mybir.dt.float8e4
AF = mybir.ActivationFunctionType
ALU = mybir.AluOpType
DR = mybir.MatmulPerfMode.DoubleRow

B, C, H, W = 8, 256, 64, 64
GROUPS = 8
EPS = 1e-5
P = 128
N_CORES = 8
ATT_SCALE = float(C) ** -0.5  # 1/16
WS = 32.0                     # host pre-scale on fp8 qkv weights
INV_WS = 1.0 / WS


def build_nc(n_tok=H * W):
    """Build the single-core Bass program (SPMD across 8 cores)."""
    CCH = C // P            # channel chunks (2)
    QT = 512                # q-tile width (one PSUM bank of f32)
    NQ = n_tok // QT        # number of q tiles (8)
    NKB = n_tok // P        # number of 128-token key blocks (32)
    NKP = NKB // 2          # key-block pairs per q tile (16)
    GSZ = C // GROUPS       # channels per group (32)

    nc = bacc.Bacc()

    # ---- DRAM I/O (per-core tensors; host shards batch over cores) ----
    x_d = nc.dram_tensor("x", [C, n_tok], F32, kind="ExternalInput")
    xf8_d = nc.dram_tensor("x_f8", [CCH, P, n_tok], FP8, kind="ExternalInput")
    qkvw_d = nc.dram_tensor("qkv_wt", [CCH, P, 3 * C], FP8, kind="ExternalInput")
    projw_d = nc.dram_tensor("proj_wt", [CCH, P, C], FP8, kind="ExternalInput")
    # packed per-partition scalars: qkb(4), projb(2), gnsc(2), gnbi(2), vb(2)
    smalls_d = nc.dram_tensor("smalls", [12, P], F32, kind="ExternalInput")
    # packed indicators: [0] = gn_ind (c -> group), [1] = gn_ind2 (group -> c)
    inds_d = nc.dram_tensor("gn_inds", [2, CCH, P, P], F32, kind="ExternalInput")
    out_d = nc.dram_tensor("out", [C, n_tok], F32, kind="ExternalOutput")

    with tile.TileContext(nc) as tc:
        with (
            tc.tile_pool(name="persist", bufs=1) as pp,
            tc.tile_pool(name="work", bufs=3) as wp,
            tc.tile_pool(name="ps", bufs=1, space="PSUM") as psp,
        ):
            # ---------------- DMAs: x_f8 first (stats + QKV input) --------
            x_f8 = pp.tile([P, CCH, n_tok], FP8, tag="x_f8")
            XPC = 4
            for pc in range(XPC):
                xs = slice(pc * (n_tok // XPC), (pc + 1) * (n_tok // XPC))
                nc.sync.dma_start(
                    x_f8[:, :, xs],
                    xf8_d.rearrange("t p n -> p t n")[:, :, xs],
                )
            qkvw = pp.tile([P, CCH, 3 * C], FP8, tag="qkvw")
            nc.sync.dma_start(qkvw[:], qkvw_d.rearrange("t p o -> p t o"))
            smalls = pp.tile([P, 12], F32, tag="smalls")
            nc.sync.dma_start(smalls[:], smalls_d.rearrange("j p -> p j"))
            qkb = smalls[:, 0:4]
            projb = smalls[:, 4:6]
            gnsc = smalls[:, 6:8]
            gnbi = smalls[:, 8:10]
            vbias = smalls[:, 10:12]        # per d-chunk [P, 1] columns
            inds = pp.tile([P, 2, CCH, P], F32, tag="inds")
            nc.sync.dma_start(inds[:], inds_d.rearrange("w t p g -> p w t g"))
            gnind2 = inds[:, 1]
            projw = pp.tile([P, CCH, C], FP8, tag="projw")
            nc.sync.dma_start(projw[:], projw_d.rearrange("t p o -> p t o"))
            # 32.0 in the den lhsT folds the 1/32 proj-weight prescale into
            # the softmax reciprocal (rec = 1/(32*den))
            ones_f8 = pp.tile([P, 2, P], FP8, tag="ones_f8")
            nc.vector.memset(ones_f8[:], 32.0)
            # 32*vbias per chunk; the V-shift constant (Wv@a + bv) is folded
            # through proj into projb_eff once the GN fold is known
            vb32 = pp.tile([P, 2], F32, tag="vb32")
            nc.vector.tensor_scalar_mul(vb32[:], vbias[:], WS)
            projb_eff = pp.tile([P, 2], F32, tag="projb_eff")

            # ---------------- GN stats (bn_stats on the fp8 x) ------------
            # quantization noise on mean/var of 128k samples is ~1e-4
            # relative -- irrelevant against the 2e-2 budget.
            stats = pp.tile([P, CCH, 2], F32, tag="stats")
            for t in range(CCH):
                bn6 = wp.tile([P, n_tok // 512, 6], F32, tag="bn6")
                xv = x_f8[:, t].rearrange("p (a b) -> p a b", b=512)
                for a in range(n_tok // 512):
                    nc.vector.bn_stats(bn6[:, a], xv[:, a])
                nc.vector.bn_aggr(stats[:, t], bn6[:])
                # stats col1 := mean^2 + var = E[x^2] (col0 stays mean)
                nc.vector.scalar_tensor_tensor(
                    out=stats[:, t, 1:2],
                    in0=stats[:, t, 0:1],
                    scalar=stats[:, t, 0:1],
                    in1=stats[:, t, 1:2],
                    op0=ALU.mult,
                    op1=ALU.add,
                )
            # f32 x for the residual: queued last, consumed from finalize()
            # well into the attention phase.
            x_sb = pp.tile([P, CCH, n_tok], F32, tag="x_sb")
            for t in range(CCH):
                for pc in range(XPC):
                    xs = slice(pc * (n_tok // XPC), (pc + 1) * (n_tok // XPC))
                    nc.sync.dma_start(x_sb[:, t, xs], x_d[t * P:(t + 1) * P, xs])

            # group aggregation: gagg[g, j] = sum_{c in group g} stats[c, j]
            gagg_ps = psp.tile([P, QT], F32, tag="p", name="gagg_ps")
            for t in range(CCH):
                nc.tensor.matmul(
                    gagg_ps[:, :2],
                    inds[:, 0, t],
                    stats[:, t],
                    start=(t == 0),
                    stop=(t == CCH - 1),
                )
            # per-group a = rstd, b = -mean * rstd. gn_ind is host-scaled by
            # 1/GSZ, so gagg cols are already (mean, E[x^2]) per group.
            gab = pp.tile([P, 2], F32, tag="gab")
            nc.vector.memset(gab[:], 0.0)
            gsb = wp.tile([P, 2], F32, tag="gsb")
            nc.vector.tensor_copy(gsb[:GROUPS], gagg_ps[:GROUPS, :2])
            gtmp = wp.tile([P, 1], F32, tag="gtmp")
            # gtmp := mean^2 - E[x^2] = -var
            nc.vector.scalar_tensor_tensor(
                out=gtmp[:GROUPS],
                in0=gsb[:GROUPS, 0:1],
                scalar=gsb[:GROUPS, 0:1],
                in1=gsb[:GROUPS, 1:2],
                op0=ALU.mult,
                op1=ALU.subtract,
            )
            # std = sqrt(-1 * gtmp + eps)
            epsb = wp.tile([P, 1], F32, tag="epsb")
            nc.vector.memset(epsb[:], EPS)
            nc.scalar.activation(gtmp[:GROUPS], gtmp[:GROUPS], AF.Sqrt,
                                 bias=epsb[:GROUPS], scale=-1.0)
            # dummy exp AFTER the sqrt (data-dependent so the scheduler
            # cannot hoist it): loads the Exp ACT table once, and both the
            # Identity copies and the real exps then run out of that table.
            dume = wp.tile([P, 1], F32, tag="dume")
            nc.scalar.activation(dume[:1], gtmp[:1], AF.Exp, scale=1.0)
            nc.vector.reciprocal(gab[:GROUPS, 0:1], gtmp[:GROUPS])  # a = rstd
            nc.vector.tensor_mul(gtmp[:GROUPS], gsb[:GROUPS, 0:1],
                                 gab[:GROUPS, 0:1])
            nc.vector.tensor_scalar_mul(gab[:GROUPS, 1:2], gtmp[:GROUPS], -1.0)

            # broadcast (a, b) to channels; fold GN into the fp8 weights:
            #   m_c = rstd_g * gamma_c ; a_c = (-mean*rstd)*gamma_c + beta_c
            #   w_eff[c, o] = qkvw[c, o] * m_c        (qkvw = 32*W^T)
            #   a2_c = 32 * a_c / m_c   (fp8; a-term via w_eff @ a2 / 1024)
            w_eff = pp.tile([P, CCH, 3 * C], FP8, tag="w_eff")
            a_col = pp.tile([P, CCH, 1], FP8, tag="a_col")
            chms = []
            for t in range(CCH):
                chab_ps = psp.tile([P, QT], F32, tag="p", name=f"chab_ps{t}")
                nc.tensor.matmul(chab_ps[:, :2], gnind2[:, t], gab[:],
                                 start=True, stop=True)
                chm = pp.tile([P, 1], F32, tag=f"chm{t}", name=f"chm{t}")
                cha = pp.tile([P, 1], F32, tag=f"cha{t}", name=f"cha{t}")
                nc.vector.tensor_mul(chm[:], chab_ps[:, 0:1], gnsc[:, t, None])
                nc.vector.scalar_tensor_tensor(
                    out=cha[:],
                    in0=chab_ps[:, 1:2],
                    scalar=gnsc[:, t, None],
                    in1=gnbi[:, t, None],
                    op0=ALU.mult,
                    op1=ALU.add,
                )
                chms.append(chm)
                nc.vector.tensor_scalar_mul(w_eff[:, t], qkvw[:, t], chm[:])
                # a2 = 32 * a_c (fp8); the Wa bias matmuls use the RAW qkvw
                # (32*W), so W@a = qkvw^T @ a2 / 1024 -- no w_eff dependency
                nc.vector.tensor_scalar_mul(a_col[:, t], cha[:], WS)

            # ---- additive-GN bias vectors via tiny matmuls on raw qkvw ----
            # j<4: qkb_eff = qkb + (W@a)      (Q/K, applied at the copies)
            # j>=4: const_col = 32*(Wv@a+bv)  (V-shift; pushed through proj
            #                                  into projb_eff below, since
            #                                  proj(out_un + c*den)/den ==
            #                                  proj(out_un)/den + proj(c))
            qkb_eff = pp.tile([P, 4], F32, tag="qkb_eff")
            const_col = pp.tile([P, CCH, 1], FP8, tag="const_col")
            for j in range(6):
                wa_ps = psp.tile([P, QT], F32, tag="p", name=f"wa_ps{j}")
                for t in range(CCH):
                    nc.tensor.matmul(
                        wa_ps[:, 0:1],
                        qkvw[:, t, j * P:(j + 1) * P],
                        a_col[:, t],
                        start=(t == 0),
                        stop=(t == CCH - 1),
                    )
                if j < 4:
                    nc.vector.scalar_tensor_tensor(
                        out=qkb_eff[:, j, None],
                        in0=wa_ps[:, 0:1],
                        scalar=1.0 / (WS * WS),
                        in1=qkb[:, j, None],
                        op0=ALU.mult,
                        op1=ALU.add,
                    )
                else:
                    nc.vector.scalar_tensor_tensor(
                        out=const_col[:, j - 4],
                        in0=wa_ps[:, 0:1],
                        scalar=1.0 / WS,
                        in1=vb32[:, j - 4, None],
                        op0=ALU.mult,
                        op1=ALU.add,
                    )
            # projb_eff = projb + Wp @ (Wv@a + bv)
            for t2 in range(CCH):
                pcv_ps = psp.tile([P, QT], F32, tag="p", name=f"pcv{t2}")
                for tc in range(CCH):
                    nc.tensor.matmul(
                        pcv_ps[:, 0:1],
                        projw[:, tc, t2 * P:(t2 + 1) * P],
                        const_col[:, tc],
                        start=(tc == 0),
                        stop=(tc == CCH - 1),
                    )
                nc.vector.scalar_tensor_tensor(
                    out=projb_eff[:, t2, None],
                    in0=pcv_ps[:, 0:1],
                    scalar=1.0 / (WS * WS),
                    in1=projb[:, t2, None],
                    op0=ALU.mult,
                    op1=ALU.add,
                )

            # ---------------- QKV ----------------
            # Q, K in (d, n) fp8; V token-major fp8. All matmuls fp8 DR
            # (K=256 contraction in one pass). Q and K live in per-512-token
            # tiles (q_tiles[h] / k_tiles[h], planes = channel chunks) so a
            # late in-loop copy of block h never creates a (false, whole-
            # tile) dependency against concurrent score reads of other
            # blocks. Early-needed copies go on ACT (idle pre-attention),
            # the rest on DVE (idle during attention).
            q_tiles = [pp.tile([P, 2, QT], FP8, tag=f"qb{h}", name=f"qb{h}")
                       for h in range(NQ)]
            k_tiles = [pp.tile([P, 2, QT], FP8, tag=f"kb{h}", name=f"kb{h}")
                       for h in range(NQ)]
            v_sb = pp.tile([P, NKB, C], FP8, tag="v_sb")

            def emit_qk_half(j, h, engine, tag=None):
                # one [P, 512] half-block of Q (j<2) or K (j>=2); j%2 is the
                # channel chunk (plane) within the per-h tile. In-loop
                # halves use the "p" bank (never the "s" rotation, which
                # would delay scores two allocations later).
                ns = slice(h * QT, (h + 1) * QT)
                dst = (q_tiles if j < 2 else k_tiles)[h][:, j % 2]
                tg, bf = tag if tag else ("p", 1)
                qp = psp.tile([P, QT], F32, tag=tg, bufs=bf,
                              name=f"qp{j}_{h}")
                nc.tensor.matmul(
                    qp[:],
                    w_eff[:, :, j * P:(j + 1) * P],
                    x_f8[:, :, ns],
                    start=True,
                    stop=True,
                    perf_mode=DR,
                )
                if engine == "act":
                    nc.scalar.activation(
                        dst, qp[:],
                        AF.Identity,
                        bias=qkb_eff[:, j, None],
                        scale=INV_WS,
                    )
                else:
                    nc.vector.tensor_scalar(
                        out=dst,
                        in0=qp[:],
                        scalar1=INV_WS,
                        scalar2=qkb_eff[:, j, None],
                        op0=ALU.mult,
                        op1=ALU.add,
                    )

            def emit_v_quad(tq):
                # four 128-token blocks of V in one 2-bank psum tile; the
                # GN/bias shift is recovered after proj, so the copies are
                # pure scaled casts split across ACT and DVE.
                vq = psp.tile([P, 4, C], F32, tag="s", bufs=2, name=f"vq{tq}")
                for k in range(4):
                    tb = 4 * tq + k
                    nc.tensor.matmul(
                        vq[:, k],
                        x_f8[:, :, tb * P:(tb + 1) * P],
                        w_eff[:, :, 2 * C:3 * C],
                        start=True,
                        stop=True,
                        perf_mode=DR,
                    )
                nc.scalar.activation(
                    v_sb[:, 4 * tq:4 * tq + 2],
                    vq[:, 0:2], AF.Identity, bias=0.0, scale=INV_WS)
                nc.vector.tensor_scalar_mul(
                    v_sb[:, 4 * tq + 2:4 * tq + 4], vq[:, 2:4], INV_WS)

            # Upfront: Q half 0, K halves 0-3 in 3-wide "o" psum batches
            # (copies alternate ACT/DVE within a batch) + one on "p";
            # all of V in 4-wide "s" quads. Streams decouple so both copy
            # engines stay busy.
            _upfront = [(2, 0), (3, 0), (0, 0), (1, 0),
                        (2, 1), (3, 1), (2, 2), (3, 2), (2, 3)]
            for b in range(3):
                tri = _upfront[3 * b:3 * b + 3]
                o3 = psp.tile([P, 3, QT], F32, tag="o", name=f"o3_{b}")
                for k, (j, h) in enumerate(tri):
                    nc.tensor.matmul(
                        o3[:, k],
                        w_eff[:, :, j * P:(j + 1) * P],
                        x_f8[:, :, h * QT:(h + 1) * QT],
                        start=True, stop=True, perf_mode=DR,
                    )
                for k, (j, h) in enumerate(tri):
                    dst = (q_tiles if j < 2 else k_tiles)[h][:, j % 2]
                    if k % 2 == 0:
                        nc.scalar.activation(
                            dst, o3[:, k], AF.Identity,
                            bias=qkb_eff[:, j, None], scale=INV_WS)
                    else:
                        nc.vector.tensor_scalar(
                            out=dst, in0=o3[:, k],
                            scalar1=INV_WS, scalar2=qkb_eff[:, j, None],
                            op0=ALU.mult, op1=ALU.add)
            emit_qk_half(3, 3, "dve", ("p", 1))
            for tq in range(NKB // 4):
                emit_v_quad(tq)

            # K halves 4-7 and Q halves 1-7 are deadline-scheduled into the
            # attention loop (K half h feeds steps 2h..2h+1 of every q tile;
            # Q half h is first read at step 16h); copies on the idle DVE.
            pending = {}

            def sched(step, fn):
                pending.setdefault(step, []).append(fn)

            for h in (4, 5, 6, 7):
                for j in (2, 3):
                    sched(2 * (h - 4) + (j - 2),
                          lambda j=j, h=h: emit_qk_half(j, h, "dve"))
            for h in range(1, 8):
                for j in (0, 1):
                    sched(16 * h - 5 + j,
                          lambda j=j, h=h: emit_qk_half(j, h, "dve"))

            # ---------------- attention ----------------
            # global steps g = qt*NKP + i ; per step:
            #   scores(g):  2 DR matmuls -> s_ps [P, 2(kb), 512]
            #   exp(g):     1 ACT instr [P, 1024] -> pt fp8
            #   pv_den(g-2): 2 PV DR matmuls + 1 ones-DR matmul into
            #                o tile [P, 3, 512] (ch0, ch1, den)
            # PV runs TWO steps behind scores so the PE never waits on the
            # scores->exp->pt chain (exp latency > PE slack per step).
            # finalize is split: rec/obs (DVE) emit at (qt, 2) BEFORE
            # pv_den(qt, 0) so the o-tile reuse is ordered; proj/res emit
            # at (qt, 3).
            o_tiles = {}
            pt_tiles = {}

            def emit_scores_exp(g):
                qt, i = divmod(g, NKP)
                s_ps = psp.tile([P, 2, QT], F32, tag="s", bufs=2,
                                name=f"s_{g}")
                for k2 in range(2):
                    kb = 2 * i + k2
                    nc.tensor.matmul(
                        s_ps[:, k2],
                        k_tiles[kb // 4][:, :, (kb % 4) * P:(kb % 4 + 1) * P],
                        q_tiles[qt][:],
                        start=True,
                        stop=True,
                        perf_mode=DR,
                    )
                pt = wp.tile([P, 2, QT], FP8, tag="pt", bufs=6, name=f"pt_{g}")
                nc.scalar.activation(
                    pt.rearrange("p a b -> p (a b)"),
                    s_ps.rearrange("p a b -> p (a b)"),
                    AF.Exp, scale=ATT_SCALE)
                pt_tiles[g] = pt

            def emit_pv(g):
                qt, i = divmod(g, NKP)
                if i == 0:
                    o_tiles[qt] = psp.tile([P, 3, QT], F32, tag="o",
                                           name=f"o_{qt}")
                o = o_tiles[qt]
                pt = pt_tiles[g]
                for ch in range(2):
                    nc.tensor.matmul(
                        o[:, ch],
                        v_sb[:, 2 * i:2 * i + 2, ch * P:(ch + 1) * P],
                        pt[:],
                        start=(i == 0),
                        stop=(i == NKP - 1),
                        perf_mode=DR,
                    )

            def emit_den(g):
                # denominator rides one step behind PV: its i==0 matmul must
                # wait for rec(qt-1) to read the previous den bank, and that
                # chain resolves ~one step later than the obs copies.
                qt, i = divmod(g, NKP)
                o = o_tiles[qt]
                pt = pt_tiles.pop(g)
                nc.tensor.matmul(
                    o[:, 2],
                    ones_f8[:],
                    pt[:],
                    start=(i == 0),
                    stop=(i == NKP - 1),
                    perf_mode=DR,
                )

            def finalize_a(qt):
                # UN-normalized fp8 copies of the PV output with the V-shift
                # constant re-applied through the den row
                # (obs = o + (c/32)*(32*den)); frees the o tile without
                # waiting on the reciprocal. Normalization happens after
                # proj: proj is linear, so proj(out/den) == proj(out)/den.
                o = o_tiles.pop(qt)
                obs = wp.tile([P, 2, QT], FP8, tag="obs", bufs=2,
                              name=f"obs{qt}")
                nc.vector.tensor_copy(obs[:, 0], o[:, 0])
                nc.vector.tensor_copy(obs[:, 1], o[:, 1])
                rec = wp.tile([P, QT], F32, tag="rec", bufs=2, name=f"rec{qt}")
                nc.vector.reciprocal_approx_fast(rec[:], o[:, 2])
                return obs, rec

            def finalize_b(qt, obs, rec, t):
                # one output-channel-chunk of proj + residual + store;
                # proj is a single fp8 DoubleRow matmul (32x weights and the
                # 1/32 both folded into rec via the 32.0 den lhsT)
                qs = slice(qt * QT, (qt + 1) * QT)
                p_ps = psp.tile([P, QT], F32, tag="p", name=f"pp_{qt}_{t}")
                nc.tensor.matmul(p_ps[:],
                                 projw[:, :, t * P:(t + 1) * P],
                                 obs[:], start=True, stop=True,
                                 perf_mode=DR)
                tmp = wp.tile([P, QT], F32, tag="tmp", bufs=2,
                              name=f"tmp{qt}_{t}")
                nc.vector.tensor_mul(tmp[:], p_ps[:], rec[:])
                res = wp.tile([P, QT], F32, tag="res", bufs=4,
                              name=f"res{qt}_{t}")
                nc.vector.scalar_tensor_tensor(
                    out=res[:],
                    in0=tmp[:],
                    scalar=projb_eff[:, t, None],
                    in1=x_sb[:, t, qs],
                    op0=ALU.add,
                    op1=ALU.add,
                )
                nc.sync.dma_start(out_d[t * P:(t + 1) * P, qs], res[:])

            NST = NQ * NKP
            obs_pending = None
            for g in range(NST):
                qt, i = divmod(g, NKP)
                emit_scores_exp(g)
                for fn in pending.pop(g, ()):
                    fn()
                if g > 2:
                    emit_den(g - 3)      # includes den(qt-1, 15) at i == 2
                if qt > 0 and i == 2:
                    obs_pending = (qt - 1,) + finalize_a(qt - 1)
                if g > 1:
                    emit_pv(g - 2)       # PV(qt, 0) waits only on the obs
                if qt > 0 and i == 5:
                    finalize_b(*obs_pending, t=0)
                if qt > 0 and i == 9:
                    finalize_b(*obs_pending, t=1)
                    obs_pending = None
            emit_pv(NST - 2)
            emit_den(NST - 3)
            emit_pv(NST - 1)
            emit_den(NST - 2)
            emit_den(NST - 1)
            oq, obs_l, rec_l = (NQ - 1,) + finalize_a(NQ - 1)
            finalize_b(oq, obs_l, rec_l, t=0)
            finalize_b(oq, obs_l, rec_l, t=1)

    nc.finalize()
    return nc


# ---------------------------------------------------------------------------
# host side
# ---------------------------------------------------------------------------

def _prep_core_inputs(inputs, n_tok=H * W):
    """Build the per-core in_maps (shared weight tensors + per-core x)."""
    CCH = C // P
    f32 = np.float32
    bf16 = ml_dtypes.bfloat16
    fp8 = mybir.dt.np(FP8)

    x = np.asarray(inputs["x"], f32).reshape(B, C, n_tok)
    gn_scale = np.asarray(inputs["gn_scale"], f32)
    gn_bias = np.asarray(inputs["gn_bias"], f32)
    qkv_w = np.asarray(inputs["qkv_w"], f32)
    qkv_b = np.asarray(inputs["qkv_b"], f32)
    proj_w = np.asarray(inputs["proj_w"], f32)
    proj_b = np.asarray(inputs["proj_b"], f32)

    qkv_wt = (np.ascontiguousarray(qkv_w.T) * WS).reshape(CCH, P, 3 * C).astype(fp8)
    proj_wt = (np.ascontiguousarray(proj_w.T) * WS).reshape(CCH, P, C).astype(fp8)

    # packed per-partition scalars: qkb(4), projb(2), gnsc(2), gnbi(2), vb(2)
    smalls = np.concatenate([
        qkv_b[:2 * C].reshape(4, P),
        proj_b.reshape(CCH, P),
        gn_scale.reshape(CCH, P),
        gn_bias.reshape(CCH, P),
        qkv_b[2 * C:].reshape(CCH, P),
    ], axis=0).astype(f32)

    ch = np.arange(C)
    gn_ind = np.zeros((CCH, P, P), f32)
    # pre-scaled by 1/GSZ: the group-aggregation matmul yields means
    gn_ind[ch // P, ch % P, ch // (C // GROUPS)] = 1.0 / (C // GROUPS)
    gn_ind2 = np.zeros((CCH, P, P), f32)
    for t in range(CCH):
        gn_ind2[t, :GROUPS, :] = (gn_ind[t, :, :GROUPS] > 0).T.astype(f32)
    gn_inds = np.stack([gn_ind, gn_ind2])

    shared = {
        "qkv_wt": qkv_wt,
        "proj_wt": proj_wt,
        "smalls": smalls,
        "gn_inds": gn_inds,
    }
    x_f8 = x.reshape(B, CCH, P, n_tok).astype(fp8)
    return [
        dict(shared, x=np.ascontiguousarray(x[i]),
             x_f8=np.ascontiguousarray(x_f8[i]))
        for i in range(B)
    ]


_NC_CACHE = {}
LAST_RESULT = None  # BassKernelResults of the most recent run (for test.py)


def _get_nc():
    if "nc" not in _NC_CACHE:
        _NC_CACHE["nc"] = build_nc()
    return _NC_CACHE["nc"]


def kernel(**inputs) -> np.ndarray:
    global LAST_RESULT
    from concourse.bass_utils import run_bass_kernel_spmd

    nc = _get_nc()
    in_maps = _prep_core_inputs(inputs)
    res = run_bass_kernel_spmd(nc, in_maps, list(range(N_CORES)))
    LAST_RESULT = res
    out = np.stack([np.asarray(res.results[i]["out"]) for i in range(B)])
    return out.reshape(B, C, H, W).astype(np.float32)
